# revision 1
# baseline (speedup 1.0000x reference)
"""GateRetention Trainium2 kernel (Bass/Tile), 8-core tensor-parallel.

Sharding: core grid (batch b = core//4, head-group g = core%4); each core owns
4 heads (512 cols of the q/k/v/g projections, 512 rows of Wo) of one batch.
RMS-norm statistics are AllReduced across each batch's 4 cores; out-proj
partials are summed on the host (row-parallel TP gather).

Precision: projections in fp32r (tf32-like); q/k/v/g staged to DRAM in fp16;
retention + out-proj in fp16 with fp32 PSUM accumulation; a 2^±10 exponent
shift on vfac/rowfac keeps the decayed v tiles inside fp16 range.

kernel(**inputs) takes the FULL inputs from reference.setup_inputs() and
returns the FULL [B, T, DIM] fp32 output.
"""
import os
import sys

sys.path.insert(0, "/opt/trn_rl_repo")

import numpy as np

import concourse.bass as bass
import concourse.bacc as bacc
import concourse.tile as tile
import concourse.mybir as mybir
from concourse import bass_utils

F32 = mybir.dt.float32
F32R = mybir.dt.float32r
F16 = mybir.dt.float16
AX = mybir.AxisListType
ALU = mybir.AluOpType
ACTF = mybir.ActivationFunctionType

B, T, DIM = 2, 4096, 2048
H, HD = 16, 128
CS = 256
NCH = T // CS              # 16 chunks
EPS = 1e-5
GLN = 16.0
SCALE = HD ** -0.5
NCORE = 8
HPC = 4                    # heads per core
PCOLS = HPC * HD           # 512 cols per core
NBLK = T // 128            # 32 token blocks of 128
VSH = 2.0 ** -2            # fp16 range shift on vv; inverse folded into rowfac

DEBUG_LVL = int(os.environ.get("GR_DEBUG", "0"))
DEBUG = bool(DEBUG_LVL)
DMASPLIT = int(os.environ.get("GR_DMASPLIT", "3"))
TRACE = bool(int(os.environ.get("GR_TRACE", "0")))

_cache = {}


def _consts_np():
    """[128, 520] fp32: identity | Lm | Om | Umask | ones."""
    ident = np.eye(128, dtype=np.float32)
    jj, ii = np.meshgrid(np.arange(128), np.arange(128), indexing="ij")
    Lm = np.where(jj <= ii, -1.0 / GLN, 0.0).astype(np.float32)
    Om = np.full((128, 128), -1.0 / GLN, np.float32)
    Um = np.where(jj <= ii, 1.0, 0.0).astype(np.float32)
    ones = np.ones((128, 8), np.float32)
    # Lc: b_i - b_mid for block0 = +1/GLN * sum_{j>i} sp_j
    Lc = np.where(jj > ii, 1.0 / GLN, 0.0).astype(np.float32)
    return np.concatenate([ident, Lm, Om, Um, ones, Lc], axis=1)


def build(debug=False):
    nc = bacc.Bacc("TRN2", target_bir_lowering=False, debug=False,
                   enable_asserts=False, num_devices=NCORE)

    # ---------------- I/O ----------------
    xT = nc.dram_tensor("xT", [DIM, T], F32R, kind="ExternalInput").ap()
    cT = nc.dram_tensor("cT", [DIM, T], F32R, kind="ExternalInput").ap()
    wq = nc.dram_tensor("wq", [DIM, PCOLS], F32R, kind="ExternalInput").ap()
    wk = nc.dram_tensor("wk", [DIM, PCOLS], F32R, kind="ExternalInput").ap()
    wv = nc.dram_tensor("wv", [DIM, PCOLS], F32R, kind="ExternalInput").ap()
    wg = nc.dram_tensor("wg", [DIM, PCOLS], F32R, kind="ExternalInput").ap()
    wgt = nc.dram_tensor("wgt", [DIM, HPC], F32R, kind="ExternalInput").ap()
    wo = nc.dram_tensor("wo", [PCOLS, DIM], F16, kind="ExternalInput").ap()
    consts = nc.dram_tensor("consts", [128, 648], F32R, kind="ExternalInput").ap()
    c16 = nc.dram_tensor("c16", [128, 128], F16, kind="ExternalInput").ap()
    out = nc.dram_tensor("out", [T, DIM], F16, kind="ExternalOutput").ap()

    def dbg(name, shape, dtype=F32):
        return nc.dram_tensor(name, shape, dtype, kind="ExternalOutput").ap()

    def _dma_gpsimd(*a, **k):
        eng = nc.gpsimd if DMASPLIT in (1, 2) else nc.sync
        return eng.dma_start(*a, **k)

    def _dma_scalar(*a, **k):
        eng = nc.scalar if DMASPLIT in (1, 3) else nc.sync
        return eng.dma_start(*a, **k)

    with tile.TileContext(nc) as tc:
        with (
            tc.tile_pool(name="const", bufs=1) as cpool,
            tc.tile_pool(name="wts", bufs=1) as wpool,
            tc.tile_pool(name="xstream", bufs=2) as xpool,
            tc.tile_pool(name="cstream", bufs=2) as ctpool,
            tc.tile_pool(name="evac", bufs=2) as epool,
            tc.tile_pool(name="persist", bufs=1) as ppool,
            tc.tile_pool(name="small", bufs=2) as spool,
            tc.tile_pool(name="ret", bufs=2) as rpool,
            tc.tile_pool(name="ps", bufs=1, space="PSUM") as psp,
            tc.tile_pool(name="dram", bufs=1, space="DRAM") as dpool,
        ):
            def ps_big():
                return psp.tile([128, 512], F32, tag="big", bufs=4, name="psbig")

            def ps_small(shape=None, dtype=F32):
                return psp.tile(shape or [128, 256], dtype, tag="small", bufs=4,
                                name="pssmall")

            # ---------------- constants ----------------
            cst = cpool.tile([128, 648], F32R, tag="consts")
            nc.sync.dma_start(cst[:], consts)
            ident = cst[:, 0:128]
            ident32 = ident.bitcast(F32)
            Lm = cst[:, 128:256]
            Om = cst[:, 256:384]
            Um = cst[:, 384:512]
            Um32 = Um.bitcast(F32)
            ones1 = cst[:, 512:513]
            Lc = cst[:, 520:648]
            i16 = cpool.tile([128, 128], F16, tag="i16")
            nc.sync.dma_start(i16[:], c16)

            # ---------------- DRAM scratch (fp16) ----------------
            if debug:
                qT_s = dbg("dbg_qT", [PCOLS, T], F16)
                kT_s = dbg("dbg_kT", [PCOLS, T], F16)
                vN_s = dbg("dbg_vN", [T, PCOLS], F16)
                gT_s = dbg("dbg_gT", [PCOLS, T], F16)
            else:
                qT_s = nc.dram_tensor("qT_s", [PCOLS, T], F16,
                                      kind="Internal").ap()
                kT_s = nc.dram_tensor("kT_s", [PCOLS, T], F16,
                                      kind="Internal").ap()
                vN_s = nc.dram_tensor("vN_s", [T, PCOLS], F16,
                                      kind="Internal").ap()
                gT_s = nc.dram_tensor("gT_s", [PCOLS, T], F16,
                                      kind="Internal").ap()
            ss_in = nc.dram_tensor("ss_in", [3, T], F32, kind="Internal").ap()
            ss_out = nc.dram_tensor("ss_out", [3, T], F32, kind="Internal").ap()

            # =========================================================
            # P1: projections (two passes), fp32r math, fp16 staging
            # =========================================================
            NT = T // 512  # 8 token n-tiles

            gtn = ppool.tile([128, NBLK, HPC], F32, tag="gtn")
            vss = ppool.tile([128, NBLK], F32, tag="vss")

            def load_w(wdram, tag):
                wt = wpool.tile([128, 16, 512], F32R, tag=tag)
                nc.sync.dma_start(
                    wt[:], wdram.rearrange("(kt p) m -> p kt m", p=128))
                return wt

            def xt_halves(n):
                tok = slice(n * 512, (n + 1) * 512)
                halves = []
                for h2 in range(2):
                    xt = xpool.tile([128, 8, 512], F32R, tag="xt")
                    nc.sync.dma_start(
                        xt[:], xT[h2 * 1024:(h2 + 1) * 1024, tok].rearrange(
                            "(kt p) m -> p kt m", p=128))
                    halves.append(xt)
                return halves

            def tproj_mms(ps, wt, xth, m):
                for k in range(16):
                    nc.tensor.matmul(
                        ps[:], wt[:, k, m * 128:(m + 1) * 128],
                        xth[k // 8][:, k % 8, :], start=(k == 0), stop=(k == 15))

            # ---------- pass A: q, k (T-layout) ----------
            wts_a = [load_w(wq, "w0"), load_w(wk, "w1")]
            for n in range(NT):
                tok = slice(n * 512, (n + 1) * 512)
                xth = xt_halves(n)
                for pi, sdram in enumerate((qT_s, kT_s)):
                    for m in range(4):
                        ps = ps_big()
                        tproj_mms(ps, wts_a[pi], xth, m)
                        ev = epool.tile([128, 512], F16, tag="ev")
                        sqt = epool.tile([128, 512], F32R, tag="sq")
                        if m % 2 == 0:
                            nc.vector.tensor_copy(ev[:], ps[:])
                            _dma_gpsimd(
                                sdram[m * 128:(m + 1) * 128, tok], ev[:])
                        else:
                            nc.scalar.copy(ev[:], ps[:])
                            _dma_scalar(
                                sdram[m * 128:(m + 1) * 128, tok], ev[:])
                        nc.scalar.activation(sqt[:], ps[:], ACTF.Square)
                        if m == 0:
                            ssps = ps_small([1, 512])
                        nc.tensor.matmul(ssps[:1, :], ones1, sqt[:],
                                         start=(m == 0), stop=(m == 3))
                        if m == 3:
                            ssev = spool.tile([1, 512], F32, tag="ssev", bufs=2)
                            nc.vector.tensor_copy(ssev[:], ssps[:1, :])
                            _dma_gpsimd(ss_in[pi:pi + 1, tok], ssev[:])

            # ---------- pass B: v natural, silu(g) T-layout, gt ----------
            wv_sb = load_w(wv, "w0")
            wg_sb = load_w(wg, "w1")
            wgt_sb = wpool.tile([128, 16, HPC], F32R, tag="wgt")
            nc.sync.dma_start(wgt_sb[:],
                              wgt.rearrange("(kt p) m -> p kt m", p=128))
            for n in range(NT):
                tok = slice(n * 512, (n + 1) * 512)
                xth = xt_halves(n)
                # v natural
                for mt in range(4):
                    msl = slice(mt * 128, (mt + 1) * 128)
                    ps = ps_big()
                    for k in range(16):
                        nc.tensor.matmul(
                            ps[:], xth[k // 8][:, k % 8, msl], wv_sb[:, k, :],
                            start=(k == 0), stop=(k == 15))
                    ev = epool.tile([128, 512], F16, tag="ev")
                    sqt = epool.tile([128, 512], F32R, tag="sq")
                    nc.scalar.activation(
                        sqt[:], ps[:], ACTF.Square,
                        accum_out=vss[:, n * 4 + mt:n * 4 + mt + 1])
                    if mt % 2 == 0:
                        nc.vector.tensor_copy(ev[:], ps[:])
                        _dma_gpsimd(
                            vN_s[n * 512 + mt * 128:n * 512 + (mt + 1) * 128, :],
                            ev[:])
                    else:
                        nc.scalar.copy(ev[:], ps[:])
                        _dma_scalar(
                            vN_s[n * 512 + mt * 128:n * 512 + (mt + 1) * 128, :],
                            ev[:])
                # silu(g), T-layout
                for m in range(4):
                    ps = ps_big()
                    tproj_mms(ps, wg_sb, xth, m)
                    ev = epool.tile([128, 512], F16, tag="ev")
                    nc.scalar.activation(ev[:], ps[:], ACTF.Silu)
                    _dma_scalar(gT_s[m * 128:(m + 1) * 128, tok], ev[:])
                # gt logits: accumulate x and c streams
                gtps = ps_small([128, 512])
                for k in range(16):
                    nc.tensor.matmul(gtps[:HPC, :], wgt_sb[:, k, :],
                                     xth[k // 8][:, k % 8, :],
                                     start=(k == 0), stop=False)
                for k in range(16):
                    ct = ctpool.tile([128, 512], F32R, tag="ct", bufs=8)
                    nc.sync.dma_start(ct[:], cT[k * 128:(k + 1) * 128, tok])
                    nc.tensor.matmul(gtps[:HPC, :], wgt_sb[:, k, :], ct[:],
                                     start=False, stop=(k == 15))
                gstg = spool.tile([HPC, 512], F32, tag="gstg", bufs=2)
                nc.vector.tensor_copy(gstg[:], gtps[:HPC, :])
                for j in range(4):
                    tp = ps_small([128, HPC])
                    nc.tensor.matmul(tp[:], gstg[:, j * 128:(j + 1) * 128],
                                     ident32[:HPC, :HPC], is_transpose=True)
                    nc.vector.tensor_copy(gtn[:, n * 4 + j, :], tp[:])

            # v sumsq: transpose [128, 32] -> [32, 128] -> ss_in row 2
            vssT = ps_small([128, 128])
            nc.tensor.matmul(vssT[:32, :], vss[:], ident32, is_transpose=True)
            vssev = spool.tile([32, 128], F32, tag="vssev", bufs=1)
            nc.vector.tensor_copy(vssev[:], vssT[:32, :])
            _dma_gpsimd(
                ss_in[2:3, :].rearrange("a (b c) -> (a b) c", c=128), vssev[:])

            # =========================================================
            # P2: AllReduce sumsq; scales; gate decays
            # =========================================================
            nc.gpsimd.collective_compute(
                "AllReduce", ALU.add,
                replica_groups=[[0, 1, 2, 3], [4, 5, 6, 7]],
                ins=[ss_in.opt()], outs=[ss_out.opt()],
            )
            ssn = ppool.tile([128, NBLK, 3], F32, tag="ssn")
            for nn_ in range(NT):
                tok = slice(nn_ * 512, (nn_ + 1) * 512)
                srt = spool.tile([3, 512], F32, tag="srt", bufs=1)
                nc.sync.dma_start(srt[:], ss_out[:, tok])
                for j in range(4):
                    tp = ps_small([128, 4])
                    nc.tensor.matmul(tp[:, :3], srt[:, j * 128:(j + 1) * 128],
                                     ident32[:3, :3], is_transpose=True)
                    nc.vector.tensor_copy(ssn[:, nn_ * 4 + j, :], tp[:, :3])
            rsn = ppool.tile([128, NBLK, 3], F32, tag="rsn")
            nc.vector.tensor_scalar(rsn[:], ssn[:], 1.0 / DIM, EPS,
                                    ALU.mult, ALU.add)
            nc.scalar.activation(rsn[:], rsn[:], ACTF.Ln)
            nc.scalar.activation(rsn[:], rsn[:], ACTF.Exp, scale=-0.5)
            skv = ppool.tile([128, NBLK], F32, tag="skv")
            nc.vector.tensor_mul(skv[:], rsn[:, :, 1], rsn[:, :, 2])
            if debug and DEBUG_LVL >= 2:
                nc.sync.dma_start(dbg("dbg_rsn", [128, NBLK * 3]),
                                  rsn[:].rearrange("p a b -> p (a b)"))

            # gate decays: sp = softplus(-z) = ln(1 + exp(-z)); -1/GLN in Lm/Om
            gtd = ppool.tile([128, NBLK, HPC], F32R, tag="gtd")
            nc.scalar.activation(gtn[:], gtn[:], ACTF.Exp, scale=-1.0)
            nc.scalar.activation(gtd[:], gtn[:], ACTF.Ln, bias=1.0)

            # per chunk: recentered b' = b - b_mid via triangular matmuls;
            # eS = exp(mid-to-mid decay) for the state recurrence
            rf = ppool.tile([128, NCH, 2, HPC], F32, tag="rf")      # rowfac
            vf = ppool.tile([128, NCH, 2, HPC], F32, tag="vf")      # vfac
            eS = ppool.tile([128, NCH, HPC], F32, tag="eS")
            for ch in range(NCH):
                b0, b1 = 2 * ch, 2 * ch + 1
                p0 = ps_small([128, HPC])
                nc.tensor.matmul(p0[:], Lc, gtd[:, b0, :], start=True, stop=True)
                p1 = ps_small([128, HPC])
                nc.tensor.matmul(p1[:], Lm, gtd[:, b1, :], start=True, stop=True)
                if ch < NCH - 1:
                    pt = ps_small([128, HPC])
                    nc.tensor.matmul(pt[:], Om, gtd[:, b1, :],
                                     start=True, stop=False)
                    nc.tensor.matmul(pt[:], Om, gtd[:, b1 + 1, :],
                                     start=False, stop=True)
                    nc.scalar.activation(eS[:, ch, :], pt[:], ACTF.Exp)
                for blk01, bps in ((0, p0), (1, p1)):
                    blk = 2 * ch + blk01
                    # rowfac = exp(b') * sq * scale / VSH
                    nc.scalar.activation(rf[:, ch, blk01, :], bps[:], ACTF.Exp)
                    nc.vector.tensor_scalar(
                        rf[:, ch, blk01, :], rf[:, ch, blk01, :],
                        rsn[:, blk, 0:1], SCALE / VSH, ALU.mult, ALU.mult)
                    # vfac = exp(-b') * sk * sv * VSH
                    nc.scalar.activation(vf[:, ch, blk01, :], bps[:], ACTF.Exp,
                                         scale=-1.0)
                    nc.vector.tensor_scalar(
                        vf[:, ch, blk01, :], vf[:, ch, blk01, :],
                        skv[:, blk:blk + 1], VSH, ALU.mult, ALU.mult)

            if debug and DEBUG_LVL >= 3:
                nc.sync.dma_start(
                    dbg("dbg_rf", [128, NCH * 2 * HPC]),
                    rf[:].rearrange("p a b c -> p (a b c)"))
                nc.sync.dma_start(
                    dbg("dbg_vf", [128, NCH * 2 * HPC]),
                    vf[:].rearrange("p a b c -> p (a b c)"))
                nc.sync.dma_start(
                    dbg("dbg_eS", [128, NCH * HPC]),
                    eS[:].rearrange("p a b -> p (a b)"))
            if debug and DEBUG_LVL >= 2:
                nc.sync.dma_start(
                    dbg("dbg_gtd", [128, NBLK * HPC]),
                    gtd[:].bitcast(F32).rearrange("p a b -> p (a b)"))

            # =========================================================
            # P3: retention + gating + out-proj, per chunk (fp16)
            # =========================================================
            if int(os.environ.get("GR_BARRIER", "0")):
                tc.prologue_barrier()
            wo_sb = wpool.tile([128, HPC, DIM], F16, tag="wo")
            nc.sync.dma_start(wo_sb[:], wo.rearrange("(h p) m -> p h m", p=128))

            S_prev = [None] * HPC
            for ch in range(NCH):
                tok = slice(ch * CS, (ch + 1) * CS)
                qc = rpool.tile([128, HPC, CS], F16, tag="qc")
                kc = rpool.tile([128, HPC, CS], F16, tag="kc")
                for t_, s_ in ((qc, qT_s), (kc, kT_s)):
                    nc.sync.dma_start(
                        t_[:], s_[:, tok].rearrange("(h p) m -> p h m", p=128))
                vcn, sg = [], []
                for blk01 in range(2):
                    bt = slice(ch * CS + blk01 * 128, ch * CS + blk01 * 128 + 128)
                    vt = rpool.tile([128, PCOLS], F16, tag="vcn")
                    nc.sync.dma_start(vt[:], vN_s[bt, :])
                    vcn.append(vt)
                    gt_ = rpool.tile([128, HPC, 128], F16, tag="gch")
                    nc.sync.dma_start(
                        gt_[:], gT_s[:, bt].rearrange("(h p) m -> p h m", p=128))
                    sg.append(gt_)
                o_st = rpool.tile([128, 2 * HPC, HD], F32, tag="o_st")
                for hl in range(HPC):
                    # k_nat via PE transpose; vv from natural v
                    knat, vvt = [], []
                    for blk01 in range(2):
                        bsl = slice(blk01 * 128, blk01 * 128 + 128)
                        if ch < NCH - 1:
                            tpk = ps_small([128, 128], F16)
                            nc.tensor.transpose(tpk[:], kc[:, hl, bsl], i16[:])
                            kn = rpool.tile([128, 128], F16, tag="knat")
                            nc.scalar.copy(kn[:], tpk[:])
                            knat.append(kn)
                        vv = rpool.tile([128, 128], F16, tag="vv")
                        nc.vector.tensor_scalar(
                            vv[:], vcn[blk01][:, hl * 128:(hl + 1) * 128],
                            vf[:, ch, blk01, hl:hl + 1], None, ALU.mult)
                        vvt.append(vv)
                    # AT (masked): rows cj, cols ci
                    at0ps = ps_small([128, 256])
                    nc.tensor.matmul(at0ps[:], kc[:, hl, 0:128], qc[:, hl, :],
                                     start=True, stop=True)
                    at0 = rpool.tile([128, CS], F16, tag="at0")
                    nc.vector.scalar_tensor_tensor(
                        at0[:, 0:128], at0ps[:, 0:128], 1.0, Um32,
                        op0=ALU.mult, op1=ALU.mult)
                    nc.scalar.copy(at0[:, 128:256], at0ps[:, 128:256])
                    at1ps = ps_small([128, 128])
                    nc.tensor.matmul(at1ps[:], kc[:, hl, 128:256],
                                     qc[:, hl, 128:256], start=True, stop=True)
                    at1 = rpool.tile([128, 128], F16, tag="at1s")
                    nc.vector.scalar_tensor_tensor(
                        at1[:], at1ps[:], 1.0, Um32, op0=ALU.mult, op1=ALU.mult)
                    # o = intra + inter (one PSUM group per ci half)
                    for ci in range(2):
                        csl = slice(ci * 128, ci * 128 + 128)
                        mms = [(at0[:, csl], vvt[0][:])]
                        if ci == 1:
                            mms.append((at1[:], vvt[1][:]))
                        if ch > 0:
                            mms.append((qc[:, hl, csl], S_prev[hl][:]))
                        ops = ps_small([128, HD])
                        for i, (lh, rh) in enumerate(mms):
                            nc.tensor.matmul(ops[:], lh, rh, start=(i == 0),
                                             stop=(i == len(mms) - 1))
                        nc.scalar.mul(o_st[:, ci * HPC + hl, :], ops[:],
                                      rf[:, ch, ci, hl:hl + 1])
                    # state update: S_cur = (S_prev + contrib) * eS
                    if ch < NCH - 1:
                        sps = ps_small([128, HD])
                        nc.tensor.matmul(sps[:], knat[0][:], vvt[0][:],
                                         start=True, stop=False)
                        nc.tensor.matmul(sps[:], knat[1][:], vvt[1][:],
                                         start=False, stop=True)
                        S_cur = rpool.tile([128, HD], F16, tag=f"S{hl}")
                        if ch > 0:
                            stmp = rpool.tile([128, HD], F32, tag="stmp")
                            nc.vector.tensor_add(stmp[:], S_prev[hl][:], sps[:])
                            nc.vector.tensor_scalar(
                                S_cur[:], stmp[:], eS[:, ch, hl:hl + 1], None,
                                ALU.mult)
                        else:
                            nc.vector.tensor_scalar(
                                S_cur[:], sps[:], eS[:, ch, hl:hl + 1], None,
                                ALU.mult)
                        S_prev[hl] = S_cur
                # o-norm over head dim (free)
                osq = rpool.tile([128, 2 * HPC, HD], F32, tag="osq", bufs=1)
                nc.scalar.activation(osq[:], o_st[:], ACTF.Square)
                ssum = rpool.tile([128, 2 * HPC], F32, tag="ossum")
                nc.vector.tensor_reduce(ssum[:], osq[:], AX.X, ALU.add)
                nc.vector.tensor_scalar(ssum[:], ssum[:], 1.0 / HD, EPS,
                                        ALU.mult, ALU.add)
                nc.vector.reciprocal(ssum[:], ssum[:])
                nc.scalar.activation(ssum[:], ssum[:], ACTF.Sqrt)
                o_n = rpool.tile([128, 2 * HPC, HD], F16, tag="o_n", bufs=2)
                nc.vector.tensor_tensor(
                    o_n[:], o_st[:],
                    ssum[:].unsqueeze(2).to_broadcast([128, 2 * HPC, HD]),
                    ALU.mult)
                # transpose + gate into go_st
                go_st = rpool.tile([128, HPC, CS], F16, tag="go_st")
                for hl in range(HPC):
                    for blk01 in range(2):
                        trp = ps_small([128, 128], F16)
                        nc.tensor.transpose(
                            trp[:], o_n[:][:, blk01 * HPC + hl, :], i16[:])
                        bsl = slice(blk01 * 128, blk01 * 128 + 128)
                        nc.vector.tensor_mul(
                            go_st[:, hl, bsl], trp[:], sg[blk01][:, hl, :])
                # out-proj for this chunk's two token tiles
                for m01 in range(2):
                    msl = slice(m01 * 128, m01 * 128 + 128)
                    for n in range(DIM // 512):
                        ps = ps_big()
                        nsl = slice(n * 512, (n + 1) * 512)
                        for k in range(HPC):
                            nc.tensor.matmul(ps[:], go_st[:, k, msl],
                                             wo_sb[:, k, nsl],
                                             start=(k == 0), stop=(k == HPC - 1))
                        oo = epool.tile([128, 512], F16, tag="oo", bufs=4)
                        if n % 2 == 0:
                            nc.vector.tensor_copy(oo[:], ps[:])
                            _dma_gpsimd(
                                out[ch * CS + m01 * 128:
                                    ch * CS + m01 * 128 + 128, nsl], oo[:])
                        else:
                            nc.scalar.copy(oo[:], ps[:])
                            _dma_scalar(
                                out[ch * CS + m01 * 128:
                                    ch * CS + m01 * 128 + 128, nsl], oo[:])

    nc.compile()
    return nc


def _prep_inputs(x, c, Wq, Wk, Wv, Wg, Wgt, Wo):
    """Build the 8 per-core input maps (host-side sharding / layout)."""
    consts = np.ascontiguousarray(_consts_np())
    c16 = np.eye(128, dtype=np.float16)
    in_maps = []
    xTs = [np.ascontiguousarray(x[b].T) for b in range(B)]
    cTs = [np.ascontiguousarray(c[b].T) for b in range(B)]
    for core in range(NCORE):
        b, g = core // 4, core % 4
        cols = slice(g * PCOLS, (g + 1) * PCOLS)
        heads = slice(g * HPC, (g + 1) * HPC)
        in_maps.append({
            "xT": xTs[b],
            "cT": cTs[b],
            "wq": np.ascontiguousarray(Wq[:, cols]),
            "wk": np.ascontiguousarray(Wk[:, cols]),
            "wv": np.ascontiguousarray(Wv[:, cols]),
            "wg": np.ascontiguousarray(Wg[:, cols]),
            "wgt": np.ascontiguousarray(Wgt[:, heads]),
            "wo": np.ascontiguousarray(Wo[cols, :]).astype(np.float16),
            "consts": consts,
            "c16": c16,
        })
    return in_maps


def kernel(x, c, Wq, Wk, Wv, Wg, Wgt, Wo, _want_results=False):
    key = "nc_dbg" if DEBUG else "nc"
    if key not in _cache:
        _cache[key] = build(debug=DEBUG)
    nc = _cache[key]
    in_maps = _prep_inputs(np.asarray(x, np.float32), np.asarray(c, np.float32),
                           np.asarray(Wq, np.float32), np.asarray(Wk, np.float32),
                           np.asarray(Wv, np.float32), np.asarray(Wg, np.float32),
                           np.asarray(Wgt, np.float32), np.asarray(Wo, np.float32))
    res = bass_utils.run_bass_kernel_spmd(
        nc, in_maps, core_ids=list(range(NCORE)), trace=TRACE)
    out = np.zeros((B, T, DIM), np.float32)
    for core in range(NCORE):
        out[core // 4] += res.results[core]["out"].astype(np.float32)
    if _want_results:
        return out, res
    return out



# revision 8
# speedup vs baseline: 1.2098x; 1.2098x over previous
"""GateRetention Trainium2 kernel (Bass/Tile), 8-core tensor-parallel.

Sharding: core grid (batch b = core//4, head-group g = core%4); each core owns
4 heads (512 cols of the q/k/v/g projections, 512 rows of Wo) of one batch.
RMS-norm statistics are AllReduced across each batch's 4 cores (two half-T
collectives so the latency hides under projection compute); out-proj partials
are summed on the host (row-parallel TP gather).

Pipeline: one merged projection pass over x (q,k,v,g,gt per 512-token tile,
fp16 operands, fp32 PSUM), fp16 staging via DRAM, then retention software-
pipelined by one chunk: retention(ch) overlaps the o-norm chain of ch-1 on
scalar/vector and the out-proj matmuls of ch-1 on PE. Elementwise work in
retention is spread over vector/scalar/gpsimd so no single engine stalls PE.

Precision: all matmuls fp16 with fp32 accumulation; a 2^-2 exponent shift on
vfac/rowfac keeps decayed v tiles inside fp16 range. x, x+c, and all weights
are cast to fp16 on the host.

kernel(**inputs) takes the FULL inputs from reference.setup_inputs() and
returns the FULL [B, T, DIM] fp32 output.
"""
import os
import sys

sys.path.insert(0, "/opt/trn_rl_repo")

import numpy as np

import concourse.bass as bass
import concourse.bacc as bacc
import concourse.tile as tile
import concourse.mybir as mybir
from concourse import bass_utils

F32 = mybir.dt.float32
F32R = mybir.dt.float32r
F16 = mybir.dt.float16
AX = mybir.AxisListType
ALU = mybir.AluOpType
ACTF = mybir.ActivationFunctionType

B, T, DIM = 2, 4096, 2048
H, HD = 16, 128
CS = 256
NCH = T // CS              # 16 chunks
EPS = 1e-5
GLN = 16.0
SCALE = HD ** -0.5
NCORE = 8
HPC = 4                    # heads per core
PCOLS = HPC * HD           # 512 cols per core
NBLK = T // 128            # 32 token blocks of 128
NT = T // 512              # 8 token n-tiles
VSH = 2.0 ** -2            # fp16 range shift on vv; inverse folded into rowfac

DEBUG_LVL = int(os.environ.get("GR_DEBUG", "0"))
DEBUG = bool(DEBUG_LVL)
TRACE = bool(int(os.environ.get("GR_TRACE", "0")))

_cache = {}


def _consts_np():
    """[128, 648] fp32: identity | Lm | Om | Umask | ones | Lc."""
    ident = np.eye(128, dtype=np.float32)
    jj, ii = np.meshgrid(np.arange(128), np.arange(128), indexing="ij")
    Lm = np.where(jj <= ii, -1.0 / GLN, 0.0).astype(np.float32)
    Om = np.full((128, 128), -1.0 / GLN, np.float32)
    Um = np.where(jj <= ii, 1.0, 0.0).astype(np.float32)
    ones = np.ones((128, 8), np.float32)
    # Lc: b_i - b_mid for block0 = +1/GLN * sum_{j>i} sp_j
    Lc = np.where(jj > ii, 1.0 / GLN, 0.0).astype(np.float32)
    return np.concatenate([ident, Lm, Om, Um, ones, Lc], axis=1)


def build(debug=False):
    nc = bacc.Bacc("TRN2", target_bir_lowering=False, debug=False,
                   enable_asserts=False, num_devices=NCORE)

    # ---------------- I/O ----------------
    xT = nc.dram_tensor("xT", [DIM, T], F16, kind="ExternalInput").ap()
    xcT = nc.dram_tensor("xcT", [DIM, T], F16, kind="ExternalInput").ap()
    wq = nc.dram_tensor("wq", [DIM, PCOLS], F16, kind="ExternalInput").ap()
    wk = nc.dram_tensor("wk", [DIM, PCOLS], F16, kind="ExternalInput").ap()
    wv = nc.dram_tensor("wv", [DIM, PCOLS], F16, kind="ExternalInput").ap()
    wg = nc.dram_tensor("wg", [DIM, PCOLS], F16, kind="ExternalInput").ap()
    wgt = nc.dram_tensor("wgt", [DIM, HPC], F16, kind="ExternalInput").ap()
    wo = nc.dram_tensor("wo", [PCOLS, DIM], F16, kind="ExternalInput").ap()
    consts = nc.dram_tensor("consts", [128, 648], F32R, kind="ExternalInput").ap()
    c16 = nc.dram_tensor("c16", [128, 136], F16, kind="ExternalInput").ap()
    out = nc.dram_tensor("out", [T, DIM], F16, kind="ExternalOutput").ap()

    def dbg(name, shape, dtype=F32):
        return nc.dram_tensor(name, shape, dtype, kind="ExternalOutput").ap()

    with tile.TileContext(nc) as tc:
        with (
            tc.tile_pool(name="const", bufs=1) as cpool,
            tc.tile_pool(name="wts", bufs=1) as wpool,
            tc.tile_pool(name="xstream", bufs=2) as xpool,
            tc.tile_pool(name="evac", bufs=2) as epool,
            tc.tile_pool(name="persist", bufs=1) as ppool,
            tc.tile_pool(name="small", bufs=2) as spool,
            tc.tile_pool(name="ret", bufs=2) as rpool,
            tc.tile_pool(name="ps", bufs=1, space="PSUM") as psp,
        ):
            def ps_big():
                return psp.tile([128, 512], F32, tag="big", bufs=4, name="psbig")

            def ps_small(shape=None, dtype=F32):
                return psp.tile(shape or [128, 256], dtype, tag="small", bufs=4,
                                name="pssmall")

            # ---------------- constants ----------------
            cst = cpool.tile([128, 648], F32R, tag="consts")
            nc.sync.dma_start(cst[:], consts)
            ident = cst[:, 0:128]
            ident32 = ident.bitcast(F32)
            Lm = cst[:, 128:256]
            Om = cst[:, 256:384]
            Um = cst[:, 384:512]
            Um32 = Um.bitcast(F32)
            ones1 = cst[:, 512:513]
            Lc = cst[:, 520:648]
            i16f = cpool.tile([128, 136], F16, tag="i16")
            nc.sync.dma_start(i16f[:], c16)
            i16 = i16f[:, 0:128]
            ones16 = i16f[:, 128:129]

            # ---------------- DRAM scratch (fp16) ----------------
            if debug:
                qT_s = dbg("dbg_qT", [PCOLS, T], F16)
                kT_s = dbg("dbg_kT", [PCOLS, T], F16)
                vN_s = dbg("dbg_vN", [T, PCOLS], F16)
                gT_s = dbg("dbg_gT", [PCOLS, T], F16)
            else:
                qT_s = nc.dram_tensor("qT_s", [PCOLS, T], F16,
                                      kind="Internal").ap()
                kT_s = nc.dram_tensor("kT_s", [PCOLS, T], F16,
                                      kind="Internal").ap()
                vN_s = nc.dram_tensor("vN_s", [T, PCOLS], F16,
                                      kind="Internal").ap()
                gT_s = nc.dram_tensor("gT_s", [PCOLS, T], F16,
                                      kind="Internal").ap()
            ss_in = [nc.dram_tensor(f"ss_in{h}", [3, T // 2], F32,
                                    kind="Internal").ap() for h in range(2)]
            ss_out = [nc.dram_tensor(f"ss_out{h}", [3, T // 2], F32,
                                     kind="Internal").ap() for h in range(2)]

            # =========================================================
            # P1: merged projection pass, fp16 math, fp16 staging
            # =========================================================
            gtn = ppool.tile([128, NBLK, HPC], F32, tag="gtn")
            vss = ppool.tile([128, NBLK], F32, tag="vss")

            # weights split per k-tile so the first matmuls start after
            # a single 128-row slice lands (kills the startup DMA stall)
            def load_w_split(wdram, tag):
                tiles = []
                for k in range(16):
                    wt = wpool.tile([128, PCOLS], F16, tag=f"{tag}{k}")
                    nc.sync.dma_start(wt[:], wdram[k * 128:(k + 1) * 128, :])
                    tiles.append(wt)
                return tiles

            def load_x_tile(n):
                tok = slice(n * 512, (n + 1) * 512)
                xts = []
                for k in range(16):
                    xt = xpool.tile([128, 512], F16, tag=f"x{k}")
                    nc.sync.dma_start(xt[:], xT[k * 128:(k + 1) * 128, tok])
                    xts.append(xt)
                return xts

            wq_t = load_w_split(wq, "wq")
            xt0 = load_x_tile(0)
            wk_t = load_w_split(wk, "wk")
            wv_t = load_w_split(wv, "wv")
            wg_t = load_w_split(wg, "wg")
            wgt_sb = wpool.tile([128, 16, HPC], F16, tag="wgt")
            nc.sync.dma_start(wgt_sb[:],
                              wgt.rearrange("(kt p) m -> p kt m", p=128))
            wo_sb = wpool.tile([128, HPC, DIM], F16, tag="wo")
            nc.sync.dma_start(wo_sb[:], wo.rearrange("(h p) m -> p h m", p=128))

            def stat_flush(pend):
                """Deferred sumsq: 4 ones-matmuls emitted one section later so
                the PE never waits on the scalar Square of a fresh PSUM."""
                if pend is None:
                    return
                sqts, pi, half, tl = pend
                ssps = ps_small([1, 512])
                for m in range(4):
                    nc.tensor.matmul(ssps[:1, :], ones16, sqts[m][:],
                                     start=(m == 0), stop=(m == 3))
                ssev = spool.tile([1, 512], F32, tag="ssev", bufs=2)
                nc.vector.tensor_copy(ssev[:], ssps[:1, :])
                nc.scalar.dma_start(ss_in[half][pi:pi + 1, tl], ssev[:])

            def proj_tile(n, xts):
                tok = slice(n * 512, (n + 1) * 512)
                half, tl = n // 4, slice((n % 4) * 512, (n % 4) * 512 + 512)
                pend = None
                # q, k: T-layout staging + deferred sumsq rows
                for pi, (wt, sdram) in enumerate(((wq_t, qT_s), (wk_t, kT_s))):
                    sqts = []
                    for m in range(4):
                        msl = slice(m * 128, (m + 1) * 128)
                        ps = ps_big()
                        for k in range(16):
                            nc.tensor.matmul(ps[:], wt[k][:, msl], xts[k][:],
                                             start=(k == 0), stop=(k == 15))
                        ev = epool.tile([128, 512], F16, tag="ev")
                        sqt = epool.tile([128, 512], F16, tag="sq", bufs=8)
                        if m % 2 == 0:
                            nc.vector.tensor_copy(ev[:], ps[:])
                            nc.sync.dma_start(sdram[msl, tok], ev[:])
                        else:
                            nc.scalar.copy(ev[:], ps[:])
                            nc.scalar.dma_start(sdram[msl, tok], ev[:])
                        nc.scalar.activation(sqt[:], ps[:], ACTF.Square)
                        sqts.append(sqt)
                    stat_flush(pend)
                    pend = (sqts, pi, half, tl)
                # v natural + sumsq accum
                for mt in range(4):
                    msl = slice(mt * 128, (mt + 1) * 128)
                    ps = ps_big()
                    for k in range(16):
                        nc.tensor.matmul(ps[:], xts[k][:, msl], wv_t[k][:],
                                         start=(k == 0), stop=(k == 15))
                    if mt == 0:
                        stat_flush(pend)
                        pend = None
                    ev = epool.tile([128, 512], F16, tag="ev")
                    sqt = epool.tile([128, 512], F16, tag="sq", bufs=8)
                    nc.scalar.activation(
                        sqt[:], ps[:], ACTF.Square,
                        accum_out=vss[:, n * 4 + mt:n * 4 + mt + 1])
                    if mt % 2 == 0:
                        nc.vector.tensor_copy(ev[:], ps[:])
                        nc.sync.dma_start(
                            vN_s[n * 512 + mt * 128:n * 512 + (mt + 1) * 128, :],
                            ev[:])
                    else:
                        nc.scalar.copy(ev[:], ps[:])
                        nc.scalar.dma_start(
                            vN_s[n * 512 + mt * 128:n * 512 + (mt + 1) * 128, :],
                            ev[:])
                # gt logits from host-precomputed (x+c)
                xc = []
                for h4 in range(4):
                    xct = xpool.tile([128, 4, 512], F16, tag="xc", bufs=4)
                    nc.sync.dma_start(
                        xct[:], xcT[h4 * 512:(h4 + 1) * 512, tok].rearrange(
                            "(kt p) m -> p kt m", p=128))
                    xc.append(xct)
                gtps = ps_small([128, 512])
                for k in range(16):
                    nc.tensor.matmul(gtps[:HPC, :], wgt_sb[:, k, :],
                                     xc[k // 4][:, k % 4, :],
                                     start=(k == 0), stop=(k == 15))
                gstg = spool.tile([HPC, 512], F32, tag="gstg", bufs=2)
                nc.vector.tensor_copy(gstg[:], gtps[:HPC, :])
                # silu(g), T-layout (gt transposes deferred past it)
                for m in range(4):
                    msl = slice(m * 128, (m + 1) * 128)
                    ps = ps_big()
                    for k in range(16):
                        nc.tensor.matmul(ps[:], wg_t[k][:, msl], xts[k][:],
                                         start=(k == 0), stop=(k == 15))
                    ev = epool.tile([128, 512], F16, tag="ev")
                    nc.scalar.activation(ev[:], ps[:], ACTF.Silu)
                    nc.scalar.dma_start(gT_s[msl, tok], ev[:])
                for j in range(4):
                    tp = ps_small([128, HPC])
                    nc.tensor.matmul(tp[:], gstg[:, j * 128:(j + 1) * 128],
                                     ident32[:HPC, :HPC], is_transpose=True)
                    nc.vector.tensor_copy(gtn[:, n * 4 + j, :], tp[:])

            def fire_allreduce(half):
                # v sumsq for this half: [128, 16] -> [16, 128] -> row
                vssT = ps_small([128, 128])
                nc.tensor.matmul(vssT[:16, :],
                                 vss[:, half * 16:(half + 1) * 16],
                                 ident32, is_transpose=True)
                vssev = spool.tile([16, 128], F32, tag="vssev", bufs=2)
                nc.vector.tensor_copy(vssev[:], vssT[:16, :])
                nc.scalar.dma_start(
                    ss_in[half][2:3, :].rearrange("a (b c) -> (a b) c", c=128),
                    vssev[:])
                nc.gpsimd.collective_compute(
                    "AllReduce", ALU.add,
                    replica_groups=[[0, 1, 2, 3], [4, 5, 6, 7]],
                    ins=[ss_in[half].opt()], outs=[ss_out[half].opt()],
                )

            for n in range(NT):
                xts = xt0 if n == 0 else load_x_tile(n)
                proj_tile(n, xts)
                if n == 3:
                    fire_allreduce(0)
            fire_allreduce(1)

            # =========================================================
            # P2a: gate decays (AllReduce-independent, PE + scalar)
            # =========================================================
            ssn = ppool.tile([128, NBLK, 3], F32, tag="ssn")
            rsn = ppool.tile([128, NBLK, 3], F32, tag="rsn")
            skv = ppool.tile([128, NBLK], F32, tag="skv")

            # gate decays: sp = softplus(-z) = ln(1 + exp(-z)); -1/GLN in Lm/Om
            gtd = ppool.tile([128, NBLK, HPC], F32R, tag="gtd")
            nc.scalar.activation(gtn[:], gtn[:], ACTF.Exp, scale=-1.0)
            nc.scalar.activation(gtd[:], gtn[:], ACTF.Ln, bias=1.0)

            # per chunk: recentered b' = b - b_mid via triangular matmuls;
            # eS = exp(mid-to-mid decay); e_rf/e_vf lack only the AR scales
            e_rf = ppool.tile([128, NCH, 2, HPC], F32, tag="erf")
            e_vf = ppool.tile([128, NCH, 2, HPC], F32, tag="evf")
            rf = ppool.tile([128, NCH, 2, HPC], F32, tag="rf")      # rowfac
            vf = ppool.tile([128, NCH, 2, HPC], F32, tag="vf")      # vfac
            eS = ppool.tile([128, NCH, HPC], F32, tag="eS")
            for ch in range(NCH):
                b0, b1 = 2 * ch, 2 * ch + 1
                p0 = ps_small([128, HPC])
                nc.tensor.matmul(p0[:], Lc, gtd[:, b0, :], start=True, stop=True)
                p1 = ps_small([128, HPC])
                nc.tensor.matmul(p1[:], Lm, gtd[:, b1, :], start=True, stop=True)
                if ch < NCH - 1:
                    pt = ps_small([128, HPC])
                    nc.tensor.matmul(pt[:], Om, gtd[:, b1, :],
                                     start=True, stop=False)
                    nc.tensor.matmul(pt[:], Om, gtd[:, b1 + 1, :],
                                     start=False, stop=True)
                    nc.scalar.activation(eS[:, ch, :], pt[:], ACTF.Exp)
                for blk01, bps in ((0, p0), (1, p1)):
                    nc.scalar.activation(e_rf[:, ch, blk01, :], bps[:], ACTF.Exp)
                    nc.scalar.activation(e_vf[:, ch, blk01, :], bps[:], ACTF.Exp,
                                         scale=-1.0)

            def scales_half(half):
                """AR-dependent: rsqrt of mean sumsq, then rf/vf for 8 chunks."""
                bsl = slice(half * 16, (half + 1) * 16)
                for nn_ in range(4):
                    tl = slice(nn_ * 512, (nn_ + 1) * 512)
                    srt = spool.tile([3, 512], F32, tag="srt", bufs=2)
                    nc.sync.dma_start(srt[:], ss_out[half][:, tl])
                    for j in range(4):
                        tp = ps_small([128, 4])
                        nc.tensor.matmul(tp[:, :3],
                                         srt[:, j * 128:(j + 1) * 128],
                                         ident32[:3, :3], is_transpose=True)
                        nc.vector.tensor_copy(
                            ssn[:, half * 16 + nn_ * 4 + j, :], tp[:, :3])
                nc.vector.tensor_scalar(rsn[:, bsl], ssn[:, bsl], 1.0 / DIM,
                                        EPS, ALU.mult, ALU.add)
                nc.scalar.activation(rsn[:, bsl], rsn[:, bsl], ACTF.Ln)
                nc.scalar.activation(rsn[:, bsl], rsn[:, bsl], ACTF.Exp,
                                     scale=-0.5)
                nc.vector.tensor_mul(skv[:, bsl], rsn[:, bsl, 1],
                                     rsn[:, bsl, 2])
                for ch in range(half * 8, half * 8 + 8):
                    for blk01 in range(2):
                        blk = 2 * ch + blk01
                        nc.vector.tensor_scalar(
                            rf[:, ch, blk01, :], e_rf[:, ch, blk01, :],
                            rsn[:, blk, 0:1], SCALE / VSH, ALU.mult, ALU.mult)
                        nc.vector.tensor_scalar(
                            vf[:, ch, blk01, :], e_vf[:, ch, blk01, :],
                            skv[:, blk:blk + 1], VSH, ALU.mult, ALU.mult)

            scales_half(0)

            if debug and DEBUG_LVL >= 2:
                nc.sync.dma_start(dbg("dbg_rsn", [128, NBLK * 3]),
                                  rsn[:].rearrange("p a b -> p (a b)"))
                nc.sync.dma_start(
                    dbg("dbg_gtd", [128, NBLK * HPC]),
                    gtd[:].bitcast(F32).rearrange("p a b -> p (a b)"))

            # =========================================================
            # P3: retention, software-pipelined by one chunk
            # =========================================================
            S_prev = [None] * HPC
            stash = {}

            def emit_loads(ch):
                tok = slice(ch * CS, (ch + 1) * CS)
                qc = rpool.tile([128, HPC, CS], F16, tag="qc", bufs=2)
                kc = rpool.tile([128, HPC, CS], F16, tag="kc", bufs=2)
                for t_, s_ in ((qc, qT_s), (kc, kT_s)):
                    nc.sync.dma_start(
                        t_[:], s_[:, tok].rearrange("(h p) m -> p h m", p=128))
                vcn, sg = [], []
                for blk01 in range(2):
                    bt = slice(ch * CS + blk01 * 128, ch * CS + blk01 * 128 + 128)
                    vt = rpool.tile([128, PCOLS], F16, tag="vcn", bufs=4)
                    nc.sync.dma_start(vt[:], vN_s[bt, :])
                    vcn.append(vt)
                    gt_ = rpool.tile([128, HPC, 128], F16, tag="gch", bufs=4)
                    nc.sync.dma_start(
                        gt_[:], gT_s[:, bt].rearrange("(h p) m -> p h m", p=128))
                    sg.append(gt_)
                return qc, kc, vcn, sg

            def emit_retention(ch, qc, kc, vcn):
                o_st = rpool.tile([128, 2 * HPC, HD], F32, tag="o_st")
                for hl in range(HPC):
                    # k_nat via PE transpose; vv from natural v
                    knat, vvt = [], []
                    for blk01 in range(2):
                        bsl = slice(blk01 * 128, blk01 * 128 + 128)
                        if ch < NCH - 1:
                            tpk = ps_small([128, 128], F16)
                            nc.tensor.transpose(tpk[:], kc[:, hl, bsl], i16[:])
                            kn = rpool.tile([128, 128], F16, tag="knat", bufs=4)
                            nc.scalar.copy(kn[:], tpk[:])
                            knat.append(kn)
                        vv = rpool.tile([128, 128], F16, tag="vv", bufs=4)
                        nc.gpsimd.tensor_scalar(
                            vv[:], vcn[blk01][:, hl * 128:(hl + 1) * 128],
                            vf[:, ch, blk01, hl:hl + 1], None, ALU.mult)
                        vvt.append(vv)
                    # AT (masked): rows cj, cols ci
                    at0ps = ps_small([128, 256])
                    nc.tensor.matmul(at0ps[:], kc[:, hl, 0:128], qc[:, hl, :],
                                     start=True, stop=True)
                    at0 = rpool.tile([128, CS], F16, tag="at0")
                    nc.vector.scalar_tensor_tensor(
                        at0[:, 0:128], at0ps[:, 0:128], 1.0, Um32,
                        op0=ALU.mult, op1=ALU.mult)
                    nc.scalar.copy(at0[:, 128:256], at0ps[:, 128:256])
                    at1ps = ps_small([128, 128])
                    nc.tensor.matmul(at1ps[:], kc[:, hl, 128:256],
                                     qc[:, hl, 128:256], start=True, stop=True)
                    at1 = rpool.tile([128, 128], F16, tag="at1s")
                    nc.vector.scalar_tensor_tensor(
                        at1[:], at1ps[:], 1.0, Um32, op0=ALU.mult, op1=ALU.mult)
                    # o = intra + inter (one PSUM group per ci half)
                    for ci in range(2):
                        csl = slice(ci * 128, ci * 128 + 128)
                        mms = [(at0[:, csl], vvt[0][:])]
                        if ci == 1:
                            mms.append((at1[:], vvt[1][:]))
                        if ch > 0:
                            mms.append((qc[:, hl, csl], S_prev[hl][:]))
                        ops = ps_small([128, HD])
                        for i, (lh, rh) in enumerate(mms):
                            nc.tensor.matmul(ops[:], lh, rh, start=(i == 0),
                                             stop=(i == len(mms) - 1))
                        nc.scalar.mul(o_st[:, ci * HPC + hl, :], ops[:],
                                      rf[:, ch, ci, hl:hl + 1])
                    # state update: S_cur = (S_prev + contrib) * eS
                    if ch < NCH - 1:
                        sps = ps_small([128, HD])
                        nc.tensor.matmul(sps[:], knat[0][:], vvt[0][:],
                                         start=True, stop=False)
                        nc.tensor.matmul(sps[:], knat[1][:], vvt[1][:],
                                         start=False, stop=(ch == 0))
                        if ch > 0:
                            # += S_prev on the PE via identity stationary
                            nc.tensor.matmul(sps[:], i16, S_prev[hl][:],
                                             start=False, stop=True)
                        S_cur = rpool.tile([128, HD], F16, tag=f"S{hl}")
                        nc.vector.tensor_scalar(
                            S_cur[:], sps[:], eS[:, ch, hl:hl + 1], None,
                            ALU.mult)
                        S_prev[hl] = S_cur
                return o_st

            def emit_norm(ch, o_st):
                # o-norm over head dim (free axis)
                osq = rpool.tile([128, 2 * HPC, HD], F16, tag="osq", bufs=1)
                ssum = rpool.tile([128, 2 * HPC], F32, tag="ossum")
                nc.scalar.activation(osq[:], o_st[:], ACTF.Square)
                nc.vector.tensor_reduce(ssum[:], osq[:], AX.X, ALU.add)
                nc.gpsimd.tensor_scalar(ssum[:], ssum[:], 1.0 / HD, EPS,
                                        ALU.mult, ALU.add)
                nc.vector.reciprocal(ssum[:], ssum[:])
                nc.scalar.activation(ssum[:], ssum[:], ACTF.Sqrt)
                o_n = rpool.tile([128, 2 * HPC, HD], F16, tag="o_n", bufs=2)
                nc.gpsimd.tensor_tensor(
                    o_n[:], o_st[:],
                    ssum[:].unsqueeze(2).to_broadcast([128, 2 * HPC, HD]),
                    ALU.mult)
                return o_n

            def emit_outproj(ch, o_n, sg):
                # transpose + gate into go_st, then out-proj
                go_st = rpool.tile([128, HPC, CS], F16, tag="go_st")
                for hl in range(HPC):
                    for blk01 in range(2):
                        trp = ps_small([128, 128], F16)
                        nc.tensor.transpose(
                            trp[:], o_n[:][:, blk01 * HPC + hl, :], i16[:])
                        bsl = slice(blk01 * 128, blk01 * 128 + 128)
                        nc.vector.tensor_mul(
                            go_st[:, hl, bsl], trp[:], sg[blk01][:, hl, :])
                for m01 in range(2):
                    msl = slice(m01 * 128, m01 * 128 + 128)
                    for n in range(DIM // 512):
                        ps = ps_big()
                        nsl = slice(n * 512, (n + 1) * 512)
                        for k in range(HPC):
                            nc.tensor.matmul(ps[:], go_st[:, k, msl],
                                             wo_sb[:, k, nsl],
                                             start=(k == 0), stop=(k == HPC - 1))
                        oo = epool.tile([128, 512], F16, tag="oo", bufs=4)
                        if n % 2 == 0:
                            nc.vector.tensor_copy(oo[:], ps[:])
                            nc.sync.dma_start(
                                out[ch * CS + m01 * 128:
                                    ch * CS + m01 * 128 + 128, nsl], oo[:])
                        else:
                            nc.scalar.copy(oo[:], ps[:])
                            nc.scalar.dma_start(
                                out[ch * CS + m01 * 128:
                                    ch * CS + m01 * 128 + 128, nsl], oo[:])

            for ch in range(NCH + 1):
                if ch < NCH:
                    qc, kc, vcn, sg = emit_loads(ch)
                    o_st = emit_retention(ch, qc, kc, vcn)
                if ch > 0:
                    emit_outproj(ch - 1, stash["o_n"], stash["sg"])
                if ch == 1:
                    scales_half(1)
                if ch < NCH:
                    o_n = emit_norm(ch, o_st)
                    stash = {"o_n": o_n, "sg": sg}

    nc.compile()
    return nc


def _prep_inputs(x, c, Wq, Wk, Wv, Wg, Wgt, Wo):
    """Build the 8 per-core input maps (host-side sharding / layout)."""
    consts = np.ascontiguousarray(_consts_np())
    c16 = np.concatenate([np.eye(128, dtype=np.float16),
                          np.ones((128, 8), np.float16)], axis=1)
    in_maps = []
    xTs = [np.ascontiguousarray(x[b].T).astype(np.float16) for b in range(B)]
    xcTs = [np.ascontiguousarray((x[b] + c[b]).T).astype(np.float16)
            for b in range(B)]
    for core in range(NCORE):
        b, g = core // 4, core % 4
        cols = slice(g * PCOLS, (g + 1) * PCOLS)
        heads = slice(g * HPC, (g + 1) * HPC)
        in_maps.append({
            "xT": xTs[b],
            "xcT": xcTs[b],
            "wq": np.ascontiguousarray(Wq[:, cols]).astype(np.float16),
            "wk": np.ascontiguousarray(Wk[:, cols]).astype(np.float16),
            "wv": np.ascontiguousarray(Wv[:, cols]).astype(np.float16),
            "wg": np.ascontiguousarray(Wg[:, cols]).astype(np.float16),
            "wgt": np.ascontiguousarray(Wgt[:, heads]).astype(np.float16),
            "wo": np.ascontiguousarray(Wo[cols, :]).astype(np.float16),
            "consts": consts,
            "c16": c16,
        })
    return in_maps


def kernel(x, c, Wq, Wk, Wv, Wg, Wgt, Wo, _want_results=False):
    key = "nc_dbg" if DEBUG else "nc"
    if key not in _cache:
        _cache[key] = build(debug=DEBUG)
    nc = _cache[key]
    in_maps = _prep_inputs(np.asarray(x, np.float32), np.asarray(c, np.float32),
                           np.asarray(Wq, np.float32), np.asarray(Wk, np.float32),
                           np.asarray(Wv, np.float32), np.asarray(Wg, np.float32),
                           np.asarray(Wgt, np.float32), np.asarray(Wo, np.float32))
    res = bass_utils.run_bass_kernel_spmd(
        nc, in_maps, core_ids=list(range(NCORE)), trace=TRACE)
    out = np.zeros((B, T, DIM), np.float32)
    for core in range(NCORE):
        out[core // 4] += res.results[core]["out"].astype(np.float32)
    if _want_results:
        return out, res
    return out


# revision 11
# speedup vs baseline: 1.2657x; 1.0462x over previous
"""GateRetention Trainium2 kernel (Bass/Tile), 8-core tensor-parallel.

Sharding: core grid (batch b = core//4, head-group g = core%4); each core owns
4 heads (512 cols of the q/k/v/g projections, 512 rows of Wo) of one batch.
RMS-norm statistics are AllReduced across each batch's 4 cores (two half-T
collectives so the latency hides under projection compute); out-proj partials
are summed on the host (row-parallel TP gather).

Pipeline: one merged projection pass over x (q,k,v,g,gt per 512-token tile,
fp16 operands, fp32 PSUM), fp16 staging via DRAM, then retention software-
pipelined by one chunk: retention(ch) overlaps the o-norm chain of ch-1 on
scalar/vector and the out-proj matmuls of ch-1 on PE. Elementwise work in
retention is spread over vector/scalar/gpsimd so no single engine stalls PE.

Precision: all matmuls fp16 with fp32 accumulation; a 2^-2 exponent shift on
vfac/rowfac keeps decayed v tiles inside fp16 range. x, x+c, and all weights
are cast to fp16 on the host.

kernel(**inputs) takes the FULL inputs from reference.setup_inputs() and
returns the FULL [B, T, DIM] fp32 output.
"""
import os
import sys

sys.path.insert(0, "/opt/trn_rl_repo")

import numpy as np

import concourse.bass as bass
import concourse.bacc as bacc
import concourse.tile as tile
import concourse.mybir as mybir
from concourse import bass_utils

F32 = mybir.dt.float32
F32R = mybir.dt.float32r
F16 = mybir.dt.float16
AX = mybir.AxisListType
ALU = mybir.AluOpType
ACTF = mybir.ActivationFunctionType

B, T, DIM = 2, 4096, 2048
H, HD = 16, 128
CS = 256
NCH = T // CS              # 16 chunks
EPS = 1e-5
GLN = 16.0
SCALE = HD ** -0.5
NCORE = 8
HPC = 4                    # heads per core
PCOLS = HPC * HD           # 512 cols per core
NBLK = T // 128            # 32 token blocks of 128
NT = T // 512              # 8 token n-tiles
VSH = 2.0 ** -2            # fp16 range shift on vv; inverse folded into rowfac

DEBUG_LVL = int(os.environ.get("GR_DEBUG", "0"))
DEBUG = bool(DEBUG_LVL)
TRACE = bool(int(os.environ.get("GR_TRACE", "0")))

_cache = {}


def _consts_np():
    """[128, 648] fp32: identity | Lm | Om | Umask | ones | Lc."""
    ident = np.eye(128, dtype=np.float32)
    jj, ii = np.meshgrid(np.arange(128), np.arange(128), indexing="ij")
    Lm = np.where(jj <= ii, -1.0 / GLN, 0.0).astype(np.float32)
    Om = np.full((128, 128), -1.0 / GLN, np.float32)
    Um = np.where(jj <= ii, 1.0, 0.0).astype(np.float32)
    ones = np.ones((128, 8), np.float32)
    # Lc: b_i - b_mid for block0 = +1/GLN * sum_{j>i} sp_j
    Lc = np.where(jj > ii, 1.0 / GLN, 0.0).astype(np.float32)
    return np.concatenate([ident, Lm, Om, Um, ones, Lc], axis=1)


def build(debug=False):
    nc = bacc.Bacc("TRN2", target_bir_lowering=False, debug=False,
                   enable_asserts=False, num_devices=NCORE)

    # ---------------- I/O ----------------
    xT = nc.dram_tensor("xT", [DIM, T], F16, kind="ExternalInput").ap()
    xcT = nc.dram_tensor("xcT", [DIM, T], F16, kind="ExternalInput").ap()
    wq = nc.dram_tensor("wq", [DIM, PCOLS], F16, kind="ExternalInput").ap()
    wk = nc.dram_tensor("wk", [DIM, PCOLS], F16, kind="ExternalInput").ap()
    wv = nc.dram_tensor("wv", [DIM, PCOLS], F16, kind="ExternalInput").ap()
    wg = nc.dram_tensor("wg", [DIM, PCOLS], F16, kind="ExternalInput").ap()
    wgt = nc.dram_tensor("wgt", [DIM, HPC], F16, kind="ExternalInput").ap()
    wo = nc.dram_tensor("wo", [PCOLS, DIM], F16, kind="ExternalInput").ap()
    consts = nc.dram_tensor("consts", [128, 648], F32R, kind="ExternalInput").ap()
    c16 = nc.dram_tensor("c16", [128, 136], F16, kind="ExternalInput").ap()
    out = nc.dram_tensor("out", [T, DIM], F16, kind="ExternalOutput").ap()

    def dbg(name, shape, dtype=F32):
        return nc.dram_tensor(name, shape, dtype, kind="ExternalOutput").ap()

    with tile.TileContext(nc) as tc:
        with (
            tc.tile_pool(name="const", bufs=1) as cpool,
            tc.tile_pool(name="wts", bufs=1) as wpool,
            tc.tile_pool(name="xstream", bufs=2) as xpool,
            tc.tile_pool(name="evac", bufs=2) as epool,
            tc.tile_pool(name="persist", bufs=1) as ppool,
            tc.tile_pool(name="small", bufs=2) as spool,
            tc.tile_pool(name="ret", bufs=2) as rpool,
            tc.tile_pool(name="ps", bufs=1, space="PSUM") as psp,
        ):
            def ps_big():
                return psp.tile([128, 512], F32, tag="big", bufs=4, name="psbig")

            def ps_small(shape=None, dtype=F32):
                return psp.tile(shape or [128, 256], dtype, tag="small", bufs=4,
                                name="pssmall")

            # ---------------- constants ----------------
            cst = cpool.tile([128, 648], F32R, tag="consts")
            nc.sync.dma_start(cst[:], consts)
            ident = cst[:, 0:128]
            ident32 = ident.bitcast(F32)
            Lm = cst[:, 128:256]
            Om = cst[:, 256:384]
            Um = cst[:, 384:512]
            Um32 = Um.bitcast(F32)
            ones1 = cst[:, 512:513]
            Lc = cst[:, 520:648]
            i16f = cpool.tile([128, 136], F16, tag="i16")
            nc.sync.dma_start(i16f[:], c16)
            i16 = i16f[:, 0:128]
            ones16 = i16f[:, 128:129]

            # ---------------- DRAM scratch (fp16) ----------------
            if debug:
                qT_s = dbg("dbg_qT", [PCOLS, T], F16)
                kT_s = dbg("dbg_kT", [PCOLS, T], F16)
                vN_s = dbg("dbg_vN", [T, PCOLS], F16)
                gT_s = dbg("dbg_gT", [PCOLS, T], F16)
            else:
                qT_s = nc.dram_tensor("qT_s", [PCOLS, T], F16,
                                      kind="Internal").ap()
                kT_s = nc.dram_tensor("kT_s", [PCOLS, T], F16,
                                      kind="Internal").ap()
                vN_s = nc.dram_tensor("vN_s", [T, PCOLS], F16,
                                      kind="Internal").ap()
                gT_s = nc.dram_tensor("gT_s", [PCOLS, T], F16,
                                      kind="Internal").ap()
            ss_in = [nc.dram_tensor(f"ss_in{h}", [3, T // 2], F32,
                                    kind="Internal").ap() for h in range(2)]
            ss_out = [nc.dram_tensor(f"ss_out{h}", [3, T // 2], F32,
                                     kind="Internal").ap() for h in range(2)]

            # =========================================================
            # P1: merged projection pass, fp16 math, fp16 staging
            # =========================================================
            gtn = ppool.tile([128, 2, NBLK // 2, HPC], F32, tag="gtn")
            vss = ppool.tile([128, NBLK], F32, tag="vss")

            # weights split per k-tile so the first matmuls start after
            # a single 128-row slice lands (kills the startup DMA stall)
            def load_w_split(wdram, tag):
                tiles = []
                for k in range(16):
                    wt = wpool.tile([128, PCOLS], F16, tag=f"{tag}{k}")
                    nc.sync.dma_start(wt[:], wdram[k * 128:(k + 1) * 128, :])
                    tiles.append(wt)
                return tiles

            def load_x_tile(n):
                tok = slice(n * 512, (n + 1) * 512)
                xts = []
                for k in range(16):
                    xt = xpool.tile([128, 512], F16, tag=f"x{k}")
                    nc.gpsimd.dma_start(xt[:], xT[k * 128:(k + 1) * 128, tok])
                    xts.append(xt)
                return xts

            wq_t = load_w_split(wq, "wq")
            xt0 = load_x_tile(0)
            wk_t = load_w_split(wk, "wk")
            wv_t = load_w_split(wv, "wv")
            wg_t = load_w_split(wg, "wg")
            wgt_sb = wpool.tile([128, 16, HPC], F16, tag="wgt")
            nc.sync.dma_start(wgt_sb[:],
                              wgt.rearrange("(kt p) m -> p kt m", p=128))
            wo_sb = wpool.tile([128, HPC, DIM], F16, tag="wo")
            nc.sync.dma_start(wo_sb[:], wo.rearrange("(h p) m -> p h m", p=128))

            def stat_flush(pend):
                """Deferred sumsq: 4 ones-matmuls emitted one section later so
                the PE never waits on the scalar Square of a fresh PSUM."""
                if pend is None:
                    return
                sqts, pi, half, tl = pend
                ssps = ps_small([1, 512])
                for m in range(4):
                    nc.tensor.matmul(ssps[:1, :], ones16, sqts[m][:],
                                     start=(m == 0), stop=(m == 3))
                ssev = spool.tile([1, 512], F32, tag="ssev", bufs=2)
                nc.vector.tensor_copy(ssev[:], ssps[:1, :])
                nc.scalar.dma_start(ss_in[half][pi:pi + 1, tl], ssev[:])

            def proj_tile(n, xts):
                tok = slice(n * 512, (n + 1) * 512)
                half, tl = n // 4, slice((n % 4) * 512, (n % 4) * 512 + 512)
                pend = None
                # q, k: T-layout staging + deferred sumsq rows
                for pi, (wt, sdram) in enumerate(((wq_t, qT_s), (wk_t, kT_s))):
                    sqts = []
                    for m in range(4):
                        msl = slice(m * 128, (m + 1) * 128)
                        ps = ps_big()
                        for k in range(16):
                            nc.tensor.matmul(ps[:], wt[k][:, msl], xts[k][:],
                                             start=(k == 0), stop=(k == 15))
                        ev = epool.tile([128, 512], F16, tag="ev")
                        sqt = epool.tile([128, 512], F16, tag="sq", bufs=8)
                        if m % 2 == 0:
                            nc.vector.tensor_copy(ev[:], ps[:])
                            nc.sync.dma_start(sdram[msl, tok], ev[:])
                        else:
                            nc.scalar.copy(ev[:], ps[:])
                            nc.scalar.dma_start(sdram[msl, tok], ev[:])
                        nc.scalar.activation(sqt[:], ps[:], ACTF.Square)
                        sqts.append(sqt)
                    stat_flush(pend)
                    pend = (sqts, pi, half, tl)
                # v natural + sumsq accum
                for mt in range(4):
                    msl = slice(mt * 128, (mt + 1) * 128)
                    ps = ps_big()
                    for k in range(16):
                        nc.tensor.matmul(ps[:], xts[k][:, msl], wv_t[k][:],
                                         start=(k == 0), stop=(k == 15))
                    if mt == 0:
                        stat_flush(pend)
                        pend = None
                    ev = epool.tile([128, 512], F16, tag="ev")
                    sqt = epool.tile([128, 512], F16, tag="sq", bufs=8)
                    nc.scalar.activation(
                        sqt[:], ps[:], ACTF.Square,
                        accum_out=vss[:, n * 4 + mt:n * 4 + mt + 1])
                    if mt % 2 == 0:
                        nc.vector.tensor_copy(ev[:], ps[:])
                        nc.sync.dma_start(
                            vN_s[n * 512 + mt * 128:n * 512 + (mt + 1) * 128, :],
                            ev[:])
                    else:
                        nc.scalar.copy(ev[:], ps[:])
                        nc.scalar.dma_start(
                            vN_s[n * 512 + mt * 128:n * 512 + (mt + 1) * 128, :],
                            ev[:])
                # gt logits from host-precomputed (x+c)
                xc = []
                for h4 in range(4):
                    xct = xpool.tile([128, 4, 512], F16, tag="xc", bufs=4)
                    nc.gpsimd.dma_start(
                        xct[:], xcT[h4 * 512:(h4 + 1) * 512, tok].rearrange(
                            "(kt p) m -> p kt m", p=128))
                    xc.append(xct)
                gtps = ps_small([128, 512])
                for k in range(16):
                    nc.tensor.matmul(gtps[:HPC, :], wgt_sb[:, k, :],
                                     xc[k // 4][:, k % 4, :],
                                     start=(k == 0), stop=(k == 15))
                gstg = spool.tile([HPC, 512], F32, tag="gstg", bufs=2)
                nc.vector.tensor_copy(gstg[:], gtps[:HPC, :])
                # silu(g), T-layout (gt transposes deferred past it)
                for m in range(4):
                    msl = slice(m * 128, (m + 1) * 128)
                    ps = ps_big()
                    for k in range(16):
                        nc.tensor.matmul(ps[:], wg_t[k][:, msl], xts[k][:],
                                         start=(k == 0), stop=(k == 15))
                    ev = epool.tile([128, 512], F16, tag="ev")
                    nc.scalar.activation(ev[:], ps[:], ACTF.Silu)
                    nc.scalar.dma_start(gT_s[msl, tok], ev[:])
                for j in range(4):
                    b = n * 4 + j
                    tp = ps_small([128, HPC])
                    nc.tensor.matmul(tp[:], gstg[:, j * 128:(j + 1) * 128],
                                     ident32[:HPC, :HPC], is_transpose=True)
                    nc.vector.tensor_copy(gtn[:, b % 2, b // 2, :], tp[:])

            def fire_allreduce(half):
                # v sumsq for this half: [128, 16] -> [16, 128] -> row
                vssT = ps_small([128, 128])
                nc.tensor.matmul(vssT[:16, :],
                                 vss[:, half * 16:(half + 1) * 16],
                                 ident32, is_transpose=True)
                vssev = spool.tile([16, 128], F32, tag="vssev", bufs=2)
                nc.vector.tensor_copy(vssev[:], vssT[:16, :])
                nc.scalar.dma_start(
                    ss_in[half][2:3, :].rearrange("a (b c) -> (a b) c", c=128),
                    vssev[:])
                nc.gpsimd.collective_compute(
                    "AllReduce", ALU.add,
                    replica_groups=[[0, 1, 2, 3], [4, 5, 6, 7]],
                    ins=[ss_in[half].opt()], outs=[ss_out[half].opt()],
                )

            for n in range(NT):
                xts = xt0 if n == 0 else load_x_tile(n)
                proj_tile(n, xts)
                if n == 3:
                    fire_allreduce(0)
            fire_allreduce(1)

            # =========================================================
            # P2a: gate decays (AllReduce-independent, PE + scalar)
            # =========================================================
            ssn = ppool.tile([128, NBLK, 3], F32, tag="ssn")
            rsn = ppool.tile([128, NBLK, 3], F32, tag="rsn")
            skv = ppool.tile([128, NBLK], F32, tag="skv")

            # gate decays: sp = softplus(-z) = ln(1 + exp(-z)); -1/GLN in Lm/Om
            gtd = ppool.tile([128, 2, NBLK // 2, HPC], F32R, tag="gtd")
            nc.scalar.activation(gtn[:], gtn[:], ACTF.Exp, scale=-1.0)
            nc.scalar.activation(gtd[:], gtn[:], ACTF.Ln, bias=1.0)

            # batched recentering: one matmul per triangular mask over all 16
            # chunks (even blocks with Lc, odd with Lm); eS accumulates the two
            # full-block sums (blocks 2ch+1, 2ch+2) in one pair of matmuls
            e_rf = ppool.tile([128, 2, NCH, HPC], F32, tag="erf")
            e_vf = ppool.tile([128, 2, NCH, HPC], F32, tag="evf")
            rf = ppool.tile([128, NCH, 2, HPC], F32, tag="rf")      # rowfac
            vf = ppool.tile([128, NCH, 2, HPC], F32, tag="vf")      # vfac
            eS = ppool.tile([128, NCH, HPC], F32, tag="eS")
            p0a = ps_small([128, NCH * HPC])
            nc.tensor.matmul(p0a[:], Lc, gtd[:, 0].rearrange("p a b -> p (a b)"),
                             start=True, stop=True)
            p1a = ps_small([128, NCH * HPC])
            nc.tensor.matmul(p1a[:], Lm, gtd[:, 1].rearrange("p a b -> p (a b)"),
                             start=True, stop=True)
            pta = ps_small([128, (NCH - 1) * HPC])
            nc.tensor.matmul(pta[:],
                             Om, gtd[:, 1, 0:NCH - 1].rearrange("p a b -> p (a b)"),
                             start=True, stop=False)
            nc.tensor.matmul(pta[:],
                             Om, gtd[:, 0, 1:NCH].rearrange("p a b -> p (a b)"),
                             start=False, stop=True)
            nc.scalar.activation(
                eS[:, 0:NCH - 1, :].rearrange("p a b -> p (a b)"), pta[:],
                ACTF.Exp)
            for blk01, bps in ((0, p0a), (1, p1a)):
                nc.scalar.activation(
                    e_rf[:, blk01].rearrange("p a b -> p (a b)"), bps[:],
                    ACTF.Exp)
                nc.scalar.activation(
                    e_vf[:, blk01].rearrange("p a b -> p (a b)"), bps[:],
                    ACTF.Exp, scale=-1.0)

            def scales_half(half):
                """AR-dependent: rsqrt of mean sumsq, then rf/vf for 8 chunks."""
                bsl = slice(half * 16, (half + 1) * 16)
                for nn_ in range(4):
                    tl = slice(nn_ * 512, (nn_ + 1) * 512)
                    srt = spool.tile([3, 512], F32, tag="srt", bufs=2)
                    nc.sync.dma_start(srt[:], ss_out[half][:, tl])
                    for j in range(4):
                        tp = ps_small([128, 4])
                        nc.tensor.matmul(tp[:, :3],
                                         srt[:, j * 128:(j + 1) * 128],
                                         ident32[:3, :3], is_transpose=True)
                        nc.vector.tensor_copy(
                            ssn[:, half * 16 + nn_ * 4 + j, :], tp[:, :3])
                nc.vector.tensor_scalar(rsn[:, bsl], ssn[:, bsl], 1.0 / DIM,
                                        EPS, ALU.mult, ALU.add)
                nc.scalar.activation(rsn[:, bsl], rsn[:, bsl], ACTF.Ln)
                nc.scalar.activation(rsn[:, bsl], rsn[:, bsl], ACTF.Exp,
                                     scale=-0.5)
                nc.vector.tensor_mul(skv[:, bsl], rsn[:, bsl, 1],
                                     rsn[:, bsl, 2])
                for ch in range(half * 8, half * 8 + 8):
                    for blk01 in range(2):
                        blk = 2 * ch + blk01
                        nc.vector.tensor_scalar(
                            rf[:, ch, blk01, :], e_rf[:, blk01, ch, :],
                            rsn[:, blk, 0:1], SCALE / VSH, ALU.mult, ALU.mult)
                        nc.vector.tensor_scalar(
                            vf[:, ch, blk01, :], e_vf[:, blk01, ch, :],
                            skv[:, blk:blk + 1], VSH, ALU.mult, ALU.mult)

            scales_half(0)

            if debug and DEBUG_LVL >= 2:
                nc.sync.dma_start(dbg("dbg_rsn", [128, NBLK * 3]),
                                  rsn[:].rearrange("p a b -> p (a b)"))
                nc.sync.dma_start(
                    dbg("dbg_gtd", [128, NBLK * HPC]),
                    gtd[:].bitcast(F32).rearrange("p a b c -> p (a b c)"))

            # =========================================================
            # P3: retention, software-pipelined by one chunk
            # =========================================================
            S_prev = [None] * HPC
            stash = {}

            def emit_loads(ch):
                tok = slice(ch * CS, (ch + 1) * CS)
                qc = rpool.tile([128, HPC, CS], F16, tag="qc", bufs=2)
                kc = rpool.tile([128, HPC, CS], F16, tag="kc", bufs=2)
                for t_, s_ in ((qc, qT_s), (kc, kT_s)):
                    nc.sync.dma_start(
                        t_[:], s_[:, tok].rearrange("(h p) m -> p h m", p=128))
                vcn, sg = [], []
                for blk01 in range(2):
                    bt = slice(ch * CS + blk01 * 128, ch * CS + blk01 * 128 + 128)
                    vt = rpool.tile([128, PCOLS], F16, tag="vcn", bufs=4)
                    nc.sync.dma_start(vt[:], vN_s[bt, :])
                    vcn.append(vt)
                    gt_ = rpool.tile([128, HPC, 128], F16, tag="gch", bufs=4)
                    nc.sync.dma_start(
                        gt_[:], gT_s[:, bt].rearrange("(h p) m -> p h m", p=128))
                    sg.append(gt_)
                return qc, kc, vcn, sg

            def emit_retention(ch, qc, kc, vcn):
                o_st = rpool.tile([128, 2 * HPC, HD], F32, tag="o_st")
                for hl in range(HPC):
                    # k_nat via PE transpose; vv from natural v
                    knat, vvt = [], []
                    for blk01 in range(2):
                        bsl = slice(blk01 * 128, blk01 * 128 + 128)
                        if ch < NCH - 1:
                            tpk = ps_small([128, 128], F16)
                            nc.tensor.transpose(tpk[:], kc[:, hl, bsl], i16[:])
                            kn = rpool.tile([128, 128], F16, tag="knat", bufs=4)
                            nc.scalar.copy(kn[:], tpk[:])
                            knat.append(kn)
                        vv = rpool.tile([128, 128], F16, tag="vv", bufs=4)
                        nc.vector.tensor_scalar(
                            vv[:], vcn[blk01][:, hl * 128:(hl + 1) * 128],
                            vf[:, ch, blk01, hl:hl + 1], None, ALU.mult)
                        vvt.append(vv)
                    # AT (masked): rows cj, cols ci
                    at0ps = ps_small([128, 256])
                    nc.tensor.matmul(at0ps[:], kc[:, hl, 0:128], qc[:, hl, :],
                                     start=True, stop=True)
                    at0 = rpool.tile([128, CS], F16, tag="at0")
                    nc.vector.scalar_tensor_tensor(
                        at0[:, 0:128], at0ps[:, 0:128], 1.0, Um32,
                        op0=ALU.mult, op1=ALU.mult)
                    nc.scalar.copy(at0[:, 128:256], at0ps[:, 128:256])
                    at1ps = ps_small([128, 128])
                    nc.tensor.matmul(at1ps[:], kc[:, hl, 128:256],
                                     qc[:, hl, 128:256], start=True, stop=True)
                    at1 = rpool.tile([128, 128], F16, tag="at1s")
                    nc.vector.scalar_tensor_tensor(
                        at1[:], at1ps[:], 1.0, Um32, op0=ALU.mult, op1=ALU.mult)
                    # o = intra + inter (one PSUM group per ci half)
                    for ci in range(2):
                        csl = slice(ci * 128, ci * 128 + 128)
                        mms = [(at0[:, csl], vvt[0][:])]
                        if ci == 1:
                            mms.append((at1[:], vvt[1][:]))
                        if ch > 0:
                            mms.append((qc[:, hl, csl], S_prev[hl][:]))
                        ops = ps_small([128, HD])
                        for i, (lh, rh) in enumerate(mms):
                            nc.tensor.matmul(ops[:], lh, rh, start=(i == 0),
                                             stop=(i == len(mms) - 1))
                        nc.scalar.mul(o_st[:, ci * HPC + hl, :], ops[:],
                                      rf[:, ch, ci, hl:hl + 1])
                    # state update: S_cur = (S_prev + contrib) * eS
                    if ch < NCH - 1:
                        sps = ps_small([128, HD])
                        nc.tensor.matmul(sps[:], knat[0][:], vvt[0][:],
                                         start=True, stop=False)
                        nc.tensor.matmul(sps[:], knat[1][:], vvt[1][:],
                                         start=False, stop=(ch == 0))
                        if ch > 0:
                            # += S_prev on the PE via identity stationary
                            nc.tensor.matmul(sps[:], i16, S_prev[hl][:],
                                             start=False, stop=True)
                        S_cur = rpool.tile([128, HD], F16, tag=f"S{hl}")
                        nc.vector.tensor_scalar(
                            S_cur[:], sps[:], eS[:, ch, hl:hl + 1], None,
                            ALU.mult)
                        S_prev[hl] = S_cur
                return o_st

            def emit_norm(ch, o_st):
                # o-norm over head dim (free axis)
                osq = rpool.tile([128, 2 * HPC, HD], F16, tag="osq", bufs=1)
                ssum = rpool.tile([128, 2 * HPC], F32, tag="ossum")
                nc.scalar.activation(osq[:], o_st[:], ACTF.Square)
                nc.vector.tensor_reduce(ssum[:], osq[:], AX.X, ALU.add)
                nc.vector.tensor_scalar(ssum[:], ssum[:], 1.0 / HD, EPS,
                                        ALU.mult, ALU.add)
                nc.vector.reciprocal(ssum[:], ssum[:])
                nc.scalar.activation(ssum[:], ssum[:], ACTF.Sqrt)
                o_n = rpool.tile([128, 2 * HPC, HD], F16, tag="o_n", bufs=2)
                nc.vector.tensor_tensor(
                    o_n[:], o_st[:],
                    ssum[:].unsqueeze(2).to_broadcast([128, 2 * HPC, HD]),
                    ALU.mult)
                return o_n

            def emit_outproj(ch, o_n, sg):
                # transpose + gate into go_st, then out-proj
                go_st = rpool.tile([128, HPC, CS], F16, tag="go_st")
                for hl in range(HPC):
                    for blk01 in range(2):
                        trp = ps_small([128, 128], F16)
                        nc.tensor.transpose(
                            trp[:], o_n[:][:, blk01 * HPC + hl, :], i16[:])
                        bsl = slice(blk01 * 128, blk01 * 128 + 128)
                        nc.vector.tensor_mul(
                            go_st[:, hl, bsl], trp[:], sg[blk01][:, hl, :])
                for m01 in range(2):
                    msl = slice(m01 * 128, m01 * 128 + 128)
                    for n in range(DIM // 512):
                        ps = ps_big()
                        nsl = slice(n * 512, (n + 1) * 512)
                        for k in range(HPC):
                            nc.tensor.matmul(ps[:], go_st[:, k, msl],
                                             wo_sb[:, k, nsl],
                                             start=(k == 0), stop=(k == HPC - 1))
                        oo = epool.tile([128, 512], F16, tag="oo", bufs=4)
                        if n % 2 == 0:
                            nc.vector.tensor_copy(oo[:], ps[:])
                            nc.sync.dma_start(
                                out[ch * CS + m01 * 128:
                                    ch * CS + m01 * 128 + 128, nsl], oo[:])
                        else:
                            nc.scalar.copy(oo[:], ps[:])
                            nc.scalar.dma_start(
                                out[ch * CS + m01 * 128:
                                    ch * CS + m01 * 128 + 128, nsl], oo[:])

            for ch in range(NCH + 1):
                if ch < NCH:
                    qc, kc, vcn, sg = emit_loads(ch)
                    o_st = emit_retention(ch, qc, kc, vcn)
                if ch > 0:
                    emit_outproj(ch - 1, stash["o_n"], stash["sg"])
                if ch == 1:
                    scales_half(1)
                if ch < NCH:
                    o_n = emit_norm(ch, o_st)
                    stash = {"o_n": o_n, "sg": sg}

    nc.compile()
    return nc


def _prep_inputs(x, c, Wq, Wk, Wv, Wg, Wgt, Wo):
    """Build the 8 per-core input maps (host-side sharding / layout)."""
    consts = np.ascontiguousarray(_consts_np())
    c16 = np.concatenate([np.eye(128, dtype=np.float16),
                          np.ones((128, 8), np.float16)], axis=1)
    in_maps = []
    xTs = [np.ascontiguousarray(x[b].T).astype(np.float16) for b in range(B)]
    xcTs = [np.ascontiguousarray((x[b] + c[b]).T).astype(np.float16)
            for b in range(B)]
    for core in range(NCORE):
        b, g = core // 4, core % 4
        cols = slice(g * PCOLS, (g + 1) * PCOLS)
        heads = slice(g * HPC, (g + 1) * HPC)
        in_maps.append({
            "xT": xTs[b],
            "xcT": xcTs[b],
            "wq": np.ascontiguousarray(Wq[:, cols]).astype(np.float16),
            "wk": np.ascontiguousarray(Wk[:, cols]).astype(np.float16),
            "wv": np.ascontiguousarray(Wv[:, cols]).astype(np.float16),
            "wg": np.ascontiguousarray(Wg[:, cols]).astype(np.float16),
            "wgt": np.ascontiguousarray(Wgt[:, heads]).astype(np.float16),
            "wo": np.ascontiguousarray(Wo[cols, :]).astype(np.float16),
            "consts": consts,
            "c16": c16,
        })
    return in_maps


def kernel(x, c, Wq, Wk, Wv, Wg, Wgt, Wo, _want_results=False):
    key = "nc_dbg" if DEBUG else "nc"
    if key not in _cache:
        _cache[key] = build(debug=DEBUG)
    nc = _cache[key]
    in_maps = _prep_inputs(np.asarray(x, np.float32), np.asarray(c, np.float32),
                           np.asarray(Wq, np.float32), np.asarray(Wk, np.float32),
                           np.asarray(Wv, np.float32), np.asarray(Wg, np.float32),
                           np.asarray(Wgt, np.float32), np.asarray(Wo, np.float32))
    res = bass_utils.run_bass_kernel_spmd(
        nc, in_maps, core_ids=list(range(NCORE)), trace=TRACE)
    out = np.zeros((B, T, DIM), np.float32)
    for core in range(NCORE):
        out[core // 4] += res.results[core]["out"].astype(np.float32)
    if _want_results:
        return out, res
    return out


# revision 29
# speedup vs baseline: 1.3898x; 1.0980x over previous
"""GateRetention Trainium2 kernel (Bass/Tile), 8-core tensor-parallel.

Sharding: core grid (batch b = core//4, head-group g = core%4); each core owns
4 heads (512 cols of the q/k/v/g projections, 512 rows of Wo) of one batch.
RMS-norm statistics are AllReduced across each batch's 4 cores (two half-T
collectives so the latency hides under projection compute); out-proj partials
are summed on the host (row-parallel TP gather).

Pipeline: one merged projection pass over x (q,k,v,g,gt per 512-token tile,
fp16 operands, fp32 PSUM), fp16 staging via DRAM, then retention software-
pipelined by one chunk: retention(ch) overlaps the o-norm chain of ch-1 on
scalar/vector and the out-proj matmuls of ch-1 on PE. Elementwise work in
retention is spread over vector/scalar/gpsimd so no single engine stalls PE.

Precision: all matmuls fp16 with fp32 accumulation; a 2^-2 exponent shift on
vfac/rowfac keeps decayed v tiles inside fp16 range. x, x+c, and all weights
are cast to fp16 on the host.

kernel(**inputs) takes the FULL inputs from reference.setup_inputs() and
returns the FULL [B, T, DIM] fp32 output.
"""
import os
import sys

sys.path.insert(0, "/opt/trn_rl_repo")

import numpy as np

import concourse.bass as bass
import concourse.bacc as bacc
import concourse.tile as tile
import concourse.mybir as mybir
from concourse import bass_utils

F32 = mybir.dt.float32
F32R = mybir.dt.float32r
F16 = mybir.dt.float16
AX = mybir.AxisListType
ALU = mybir.AluOpType
ACTF = mybir.ActivationFunctionType

B, T, DIM = 2, 4096, 2048
H, HD = 16, 128
CS = 256
NCH = T // CS              # 16 chunks
EPS = 1e-5
GLN = 16.0
SCALE = HD ** -0.5
NCORE = 8
HPC = 4                    # heads per core
PCOLS = HPC * HD           # 512 cols per core
NBLK = T // 128            # 32 token blocks of 128
NT = T // 512              # 8 token n-tiles
VSH = 2.0 ** -2            # fp16 range shift on vv; inverse folded into rowfac

DEBUG_LVL = int(os.environ.get("GR_DEBUG", "0"))
DEBUG = bool(DEBUG_LVL)
TRACE = bool(int(os.environ.get("GR_TRACE", "0")))

_cache = {}


def _consts_np():
    """[128, 650] fp32: identity | Lm | Om | Um | ones | Lc | biases."""
    ident = np.eye(128, dtype=np.float32)
    jj, ii = np.meshgrid(np.arange(128), np.arange(128), indexing="ij")
    Lm = np.where(jj <= ii, -1.0 / GLN, 0.0).astype(np.float32)
    Om = np.full((128, 128), -1.0 / GLN, np.float32)
    Um = np.where(jj <= ii, 1.0, 0.0).astype(np.float32)
    ones = np.ones((128, 8), np.float32)
    # Lc: b_i - b_mid for block0 = +1/GLN * sum_{j>i} sp_j
    Lc = np.where(jj > ii, 1.0 / GLN, 0.0).astype(np.float32)
    bias = np.tile(np.array([[LN_RF, LN_VF]], np.float32), (128, 1))
    return np.concatenate([ident, Lm, Om, Um, ones, Lc, bias], axis=1)


def build(debug=False):
    nc = bacc.Bacc("TRN2", target_bir_lowering=False, debug=False,
                   enable_asserts=False, num_devices=NCORE)

    # ---------------- I/O ----------------
    xT = nc.dram_tensor("xT", [DIM, T], F16, kind="ExternalInput").ap()
    xcT = nc.dram_tensor("xcT", [DIM, T], F16, kind="ExternalInput").ap()
    wq = nc.dram_tensor("wq", [DIM, PCOLS], F16, kind="ExternalInput").ap()
    wk = nc.dram_tensor("wk", [DIM, PCOLS], F16, kind="ExternalInput").ap()
    wv = nc.dram_tensor("wv", [DIM, PCOLS], F16, kind="ExternalInput").ap()
    wg = nc.dram_tensor("wg", [DIM, PCOLS], F16, kind="ExternalInput").ap()
    wgt = nc.dram_tensor("wgt", [DIM, HPC], F16, kind="ExternalInput").ap()
    wo = nc.dram_tensor("wo", [PCOLS, DIM], F16, kind="ExternalInput").ap()
    consts = nc.dram_tensor("consts", [128, 650], F32R, kind="ExternalInput").ap()
    c16 = nc.dram_tensor("c16", [128, 136], F16, kind="ExternalInput").ap()
    out = nc.dram_tensor("out", [T, DIM], F16, kind="ExternalOutput").ap()

    def dbg(name, shape, dtype=F32):
        return nc.dram_tensor(name, shape, dtype, kind="ExternalOutput").ap()

    with tile.TileContext(nc) as tc:
        with (
            tc.tile_pool(name="const", bufs=1) as cpool,
            tc.tile_pool(name="wts", bufs=1) as wpool,
            tc.tile_pool(name="xstream", bufs=2) as xpool,
            tc.tile_pool(name="evac", bufs=2) as epool,
            tc.tile_pool(name="persist", bufs=1) as ppool,
            tc.tile_pool(name="small", bufs=2) as spool,
            tc.tile_pool(name="ret", bufs=2) as rpool,
            tc.tile_pool(name="ps", bufs=1, space="PSUM") as psp,
        ):
            def ps_big():
                return psp.tile([128, 512], F32, tag="big", bufs=4, name="psbig")

            def ps_small(shape=None, dtype=F32):
                return psp.tile(shape or [128, 256], dtype, tag="small", bufs=4,
                                name="pssmall")

            # ---------------- constants ----------------
            cst = cpool.tile([128, 650], F32R, tag="consts")
            nc.sync.dma_start(cst[:], consts)
            ident = cst[:, 0:128]
            ident32 = ident.bitcast(F32)
            Lm = cst[:, 128:256]
            Om = cst[:, 256:384]
            Um = cst[:, 384:512]
            Um32 = Um.bitcast(F32)
            ones1 = cst[:, 512:513]
            Lc = cst[:, 520:648]
            ln_rf = cst[:, 648:649].bitcast(F32)
            ln_vf = cst[:, 649:650].bitcast(F32)
            i16f = cpool.tile([128, 136], F16, tag="i16")
            nc.sync.dma_start(i16f[:], c16)
            i16 = i16f[:, 0:128]
            ones16 = i16f[:, 128:129]

            # ---------------- DRAM scratch (fp16) ----------------
            if debug:
                qT_s = dbg("dbg_qT", [PCOLS, T], F16)
                kT_s = dbg("dbg_kT", [PCOLS, T], F16)
                vN_s = dbg("dbg_vN", [T, PCOLS], F16)
                gT_s = dbg("dbg_gT", [PCOLS, T], F16)
            else:
                qT_s = nc.dram_tensor("qT_s", [PCOLS, T], F16,
                                      kind="Internal").ap()
                kT_s = nc.dram_tensor("kT_s", [PCOLS, T], F16,
                                      kind="Internal").ap()
                vN_s = nc.dram_tensor("vN_s", [T, PCOLS], F16,
                                      kind="Internal").ap()
                gT_s = nc.dram_tensor("gT_s", [PCOLS, T], F16,
                                      kind="Internal").ap()
            ss_in = [nc.dram_tensor(f"ss_in{h}", [3, T // 2], F32,
                                    kind="Internal").ap() for h in range(2)]
            ss_out = [nc.dram_tensor(f"ss_out{h}", [3, T // 2], F32,
                                     kind="Internal").ap() for h in range(2)]

            # =========================================================
            # P1: merged projection pass, fp16 math, fp16 staging
            # =========================================================
            gtn = ppool.tile([128, 2, NBLK // 2, HPC], F32, tag="gtn")
            vss = ppool.tile([128, NBLK], F32, tag="vss")

            # weights split per k-tile so the first matmuls start after
            # a single 128-row slice lands (kills the startup DMA stall)
            def load_w_split(wdram, tag):
                tiles = []
                for k in range(16):
                    wt = wpool.tile([128, PCOLS], F16, tag=f"{tag}{k}")
                    nc.sync.dma_start(wt[:], wdram[k * 128:(k + 1) * 128, :])
                    tiles.append(wt)
                return tiles

            def load_x_tile(n):
                tok = slice(n * 512, (n + 1) * 512)
                xts = []
                for k in range(16):
                    xt = xpool.tile([128, 512], F16, tag=f"x{k}")
                    nc.gpsimd.dma_start(xt[:], xT[k * 128:(k + 1) * 128, tok])
                    xts.append(xt)
                return xts

            wq_t = load_w_split(wq, "wq")
            xt0 = load_x_tile(0)
            wk_t = load_w_split(wk, "wk")
            wv_t = load_w_split(wv, "wv")
            wg_t = load_w_split(wg, "wg")
            wgt_sb = wpool.tile([128, 16, HPC], F16, tag="wgt")
            nc.sync.dma_start(wgt_sb[:],
                              wgt.rearrange("(kt p) m -> p kt m", p=128))
            wo_sb = wpool.tile([128, HPC, DIM], F16, tag="wo")
            nc.sync.dma_start(wo_sb[:], wo.rearrange("(h p) m -> p h m", p=128))

            def stat_flush(pend):
                """Deferred sumsq: 4 ones-matmuls emitted one section later so
                the PE never waits on the scalar Square of a fresh PSUM."""
                if pend is None:
                    return
                sqts, pi, half, tl = pend
                ssps = ps_small([1, 512])
                for m in range(4):
                    nc.tensor.matmul(ssps[:1, :], ones16, sqts[m][:],
                                     start=(m == 0), stop=(m == 3))
                ssev = spool.tile([1, 512], F32, tag="ssev", bufs=2)
                nc.vector.tensor_copy(ssev[:], ssps[:1, :])
                nc.scalar.dma_start(ss_in[half][pi:pi + 1, tl], ssev[:])

            def gt_matmuls(n):
                tok = slice(n * 512, (n + 1) * 512)
                xc = []
                for h4 in range(4):
                    xct = xpool.tile([128, 4, 512], F16, tag="xc", bufs=4)
                    nc.gpsimd.dma_start(
                        xct[:], xcT[h4 * 512:(h4 + 1) * 512, tok].rearrange(
                            "(kt p) m -> p kt m", p=128))
                    xc.append(xct)
                gtps = ps_small([128, 512])
                for k in range(16):
                    nc.tensor.matmul(gtps[:HPC, :], wgt_sb[:, k, :],
                                     xc[k // 4][:, k % 4, :],
                                     start=(k == 0), stop=(k == 15))
                gstg = spool.tile([HPC, 512], F32, tag="gstg", bufs=2)
                nc.vector.tensor_copy(gstg[:], gtps[:HPC, :])
                return gstg

            def gt_transposes(n, gstg):
                for j in range(4):
                    b = n * 4 + j
                    tp = ps_small([128, HPC])
                    nc.tensor.matmul(tp[:], gstg[:, j * 128:(j + 1) * 128],
                                     ident32[:HPC, :HPC], is_transpose=True)
                    nc.vector.tensor_copy(gtn[:, b % 2, b // 2, :], tp[:])

            def proj_tile(n, xts, with_gt=True):
                tok = slice(n * 512, (n + 1) * 512)
                half, tl = n // 4, slice((n % 4) * 512, (n % 4) * 512 + 512)
                pend = None

                def qk_evac(ps, m, sdram, sqts):
                    msl = slice(m * 128, (m + 1) * 128)
                    ev = epool.tile([128, 512], F16, tag="ev", bufs=4)
                    sqt = epool.tile([128, 512], F16, tag="sq", bufs=8)
                    if m % 2 == 0:
                        nc.vector.tensor_copy(ev[:], ps[:])
                        nc.sync.dma_start(sdram[msl, tok], ev[:])
                    else:
                        nc.scalar.copy(ev[:], ps[:])
                        nc.gpsimd.dma_start(sdram[msl, tok], ev[:])
                    nc.scalar.activation(sqt[:], ps[:], ACTF.Square)
                    sqts.append(sqt)

                # q, k: T-layout staging + deferred sumsq rows
                for pi, (wt, sdram) in enumerate(((wq_t, qT_s), (wk_t, kT_s))):
                    sqts = []
                    if n == 0 and pi == 0:
                        # cold start: k-outer so the first matmul begins after
                        # one 128-row weight/x slice lands, not the full tile
                        pss = [ps_big() for _ in range(4)]
                        for k in range(16):
                            for m in range(4):
                                nc.tensor.matmul(
                                    pss[m][:], wt[k][:, m * 128:(m + 1) * 128],
                                    xts[k][:], start=(k == 0), stop=(k == 15),
                                    skip_group_check=True)
                            if k % 4 == 3:
                                yield
                        for m in range(4):
                            qk_evac(pss[m], m, sdram, sqts)
                    else:
                        for m in range(4):
                            ps = ps_big()
                            for k in range(16):
                                nc.tensor.matmul(ps[:], wt[k][:, m * 128:
                                                               (m + 1) * 128],
                                                 xts[k][:],
                                                 start=(k == 0), stop=(k == 15))
                            qk_evac(ps, m, sdram, sqts)
                            yield
                    stat_flush(pend)
                    pend = (sqts, pi, half, tl)
                # v natural + sumsq accum
                for mt in range(4):
                    msl = slice(mt * 128, (mt + 1) * 128)
                    ps = ps_big()
                    for k in range(16):
                        nc.tensor.matmul(ps[:], xts[k][:, msl], wv_t[k][:],
                                         start=(k == 0), stop=(k == 15))
                    if mt == 0:
                        stat_flush(pend)
                        pend = None
                    ev = epool.tile([128, 512], F16, tag="ev", bufs=4)
                    sqt = epool.tile([128, 512], F16, tag="sq", bufs=8)
                    nc.scalar.activation(
                        sqt[:], ps[:], ACTF.Square,
                        accum_out=vss[:, n * 4 + mt:n * 4 + mt + 1])
                    if mt % 2 == 0:
                        nc.vector.tensor_copy(ev[:], ps[:])
                        nc.sync.dma_start(
                            vN_s[n * 512 + mt * 128:n * 512 + (mt + 1) * 128, :],
                            ev[:])
                    else:
                        nc.scalar.copy(ev[:], ps[:])
                        nc.gpsimd.dma_start(
                            vN_s[n * 512 + mt * 128:n * 512 + (mt + 1) * 128, :],
                            ev[:])
                    yield
                if n == 7:
                    fire_allreduce(1)
                if with_gt:
                    gstg = gt_matmuls(n)
                    yield
                # silu(g), T-layout (gt transposes deferred past it)
                for m in range(4):
                    msl = slice(m * 128, (m + 1) * 128)
                    ps = ps_big()
                    for k in range(16):
                        nc.tensor.matmul(ps[:], wg_t[k][:, msl], xts[k][:],
                                         start=(k == 0), stop=(k == 15))
                    ev = epool.tile([128, 512], F16, tag="ev", bufs=4)
                    nc.scalar.activation(ev[:], ps[:], ACTF.Silu)
                    nc.scalar.dma_start(gT_s[msl, tok], ev[:])
                    yield
                if with_gt:
                    gt_transposes(n, gstg)
                yield

            def fire_allreduce(half):
                # v sumsq for this half: [128, 16] -> [16, 128] -> row
                vssT = ps_small([128, 128])
                nc.tensor.matmul(vssT[:16, :],
                                 vss[:, half * 16:(half + 1) * 16],
                                 ident32, is_transpose=True)
                vssev = spool.tile([16, 128], F32, tag="vssev", bufs=2)
                nc.vector.tensor_copy(vssev[:], vssT[:16, :])
                nc.scalar.dma_start(
                    ss_in[half][2:3, :].rearrange("a (b c) -> (a b) c", c=128),
                    vssev[:])
                nc.gpsimd.collective_compute(
                    "AllReduce", ALU.add,
                    replica_groups=[[0, 1, 2, 3], [4, 5, 6, 7]],
                    ins=[ss_in[half].opt()], outs=[ss_out[half].opt()],
                )

            for n in range(NT):
                xts = xt0 if n == 0 else load_x_tile(n)
                proj_tile(n, xts)
                if n == 3:
                    fire_allreduce(0)
            fire_allreduce(1)

            # =========================================================
            # P2a: gate decays (AllReduce-independent, PE + scalar)
            # =========================================================
            ssn = ppool.tile([128, NBLK, 3], F32, tag="ssn")
            rsn = ppool.tile([128, NBLK, 3], F32, tag="rsn")
            skv = ppool.tile([128, NBLK], F32, tag="skv")

            # gate decays: sp = softplus(-z) = ln(1 + exp(-z)); -1/GLN in Lm/Om
            gtd = ppool.tile([128, 2, NBLK // 2, HPC], F32R, tag="gtd")
            nc.scalar.activation(gtn[:], gtn[:], ACTF.Exp, scale=-1.0)
            nc.scalar.activation(gtd[:], gtn[:], ACTF.Ln, bias=1.0)

            # batched recentering: one matmul per triangular mask over all 16
            # chunks (even blocks with Lc, odd with Lm); eS accumulates the two
            # full-block sums (blocks 2ch+1, 2ch+2) in one pair of matmuls
            e_rf = ppool.tile([128, 2, NCH, HPC], F32, tag="erf")
            e_vf = ppool.tile([128, 2, NCH, HPC], F32, tag="evf")
            rf = ppool.tile([128, NCH, 2, HPC], F32, tag="rf")      # rowfac
            vf = ppool.tile([128, NCH, 2, HPC], F32, tag="vf")      # vfac
            eS = ppool.tile([128, NCH, HPC], F32, tag="eS")
            p0a = ps_small([128, NCH * HPC])
            nc.tensor.matmul(p0a[:], Lc, gtd[:, 0].rearrange("p a b -> p (a b)"),
                             start=True, stop=True)
            p1a = ps_small([128, NCH * HPC])
            nc.tensor.matmul(p1a[:], Lm, gtd[:, 1].rearrange("p a b -> p (a b)"),
                             start=True, stop=True)
            pta = ps_small([128, (NCH - 1) * HPC])
            nc.tensor.matmul(pta[:],
                             Om, gtd[:, 1, 0:NCH - 1].rearrange("p a b -> p (a b)"),
                             start=True, stop=False)
            nc.tensor.matmul(pta[:],
                             Om, gtd[:, 0, 1:NCH].rearrange("p a b -> p (a b)"),
                             start=False, stop=True)
            nc.scalar.activation(
                eS[:, 0:NCH - 1, :].rearrange("p a b -> p (a b)"), pta[:],
                ACTF.Exp)
            for blk01, bps in ((0, p0a), (1, p1a)):
                nc.scalar.activation(
                    e_rf[:, blk01].rearrange("p a b -> p (a b)"), bps[:],
                    ACTF.Exp)
                nc.scalar.activation(
                    e_vf[:, blk01].rearrange("p a b -> p (a b)"), bps[:],
                    ACTF.Exp, scale=-1.0)

            def scales_half(half):
                """AR-dependent: rsqrt of mean sumsq, then rf/vf for 8 chunks."""
                bsl = slice(half * 16, (half + 1) * 16)
                for nn_ in range(4):
                    tl = slice(nn_ * 512, (nn_ + 1) * 512)
                    srt = spool.tile([3, 512], F32, tag="srt", bufs=2)
                    nc.sync.dma_start(srt[:], ss_out[half][:, tl])
                    for j in range(4):
                        tp = ps_small([128, 4])
                        nc.tensor.matmul(tp[:, :3],
                                         srt[:, j * 128:(j + 1) * 128],
                                         ident32[:3, :3], is_transpose=True)
                        nc.vector.tensor_copy(
                            ssn[:, half * 16 + nn_ * 4 + j, :], tp[:, :3])
                nc.vector.tensor_scalar(rsn[:, bsl], ssn[:, bsl], 1.0 / DIM,
                                        EPS, ALU.mult, ALU.add)
                nc.scalar.activation(rsn[:, bsl], rsn[:, bsl], ACTF.Ln)
                nc.scalar.activation(rsn[:, bsl], rsn[:, bsl], ACTF.Exp,
                                     scale=-0.5)
                nc.vector.tensor_mul(skv[:, bsl], rsn[:, bsl, 1],
                                     rsn[:, bsl, 2])
                for ch in range(half * 8, half * 8 + 8):
                    for blk01 in range(2):
                        blk = 2 * ch + blk01
                        nc.vector.tensor_scalar(
                            rf[:, ch, blk01, :], e_rf[:, blk01, ch, :],
                            rsn[:, blk, 0:1], SCALE / VSH, ALU.mult, ALU.mult)
                        nc.vector.tensor_scalar(
                            vf[:, ch, blk01, :], e_vf[:, blk01, ch, :],
                            skv[:, blk:blk + 1], VSH, ALU.mult, ALU.mult)

            scales_half(0)

            if debug and DEBUG_LVL >= 2:
                nc.sync.dma_start(dbg("dbg_rsn", [128, NBLK * 3]),
                                  rsn[:].rearrange("p a b -> p (a b)"))
                nc.sync.dma_start(
                    dbg("dbg_gtd", [128, NBLK * HPC]),
                    gtd[:].bitcast(F32).rearrange("p a b c -> p (a b c)"))

            # =========================================================
            # P3: retention, software-pipelined by one chunk
            # =========================================================
            S_prev = [None] * HPC

            def emit_loads(ch):
                tok = slice(ch * CS, (ch + 1) * CS)
                qc = rpool.tile([128, HPC, CS], F16, tag="qc", bufs=2)
                kc = rpool.tile([128, HPC, CS], F16, tag="kc", bufs=2)
                for t_, s_ in ((qc, qT_s), (kc, kT_s)):
                    nc.sync.dma_start(
                        t_[:], s_[:, tok].rearrange("(h p) m -> p h m", p=128))
                vcn, sg = [], []
                for blk01 in range(2):
                    bt = slice(ch * CS + blk01 * 128, ch * CS + blk01 * 128 + 128)
                    vt = rpool.tile([128, PCOLS], F16, tag="vcn", bufs=4)
                    nc.sync.dma_start(vt[:], vN_s[bt, :])
                    vcn.append(vt)
                    gt_ = rpool.tile([128, HPC, 128], F16, tag="gch", bufs=4)
                    nc.sync.dma_start(
                        gt_[:], gT_s[:, bt].rearrange("(h p) m -> p h m", p=128))
                    sg.append(gt_)
                return qc, kc, vcn, sg

            def emit_produce(ch, hl, qc, kc):
                """Per-head producers: k transposes + AT matmuls (PE) and
                their fp16 evacs with vfac folded in (scalar/vector). Runs one
                head ahead of emit_consume so PE never waits on the evacs."""
                knat = []
                for blk01 in range(2):
                    bsl = slice(blk01 * 128, blk01 * 128 + 128)
                    if ch < NCH - 1:
                        tpk = ps_small([128, 128], F16)
                        nc.tensor.transpose(tpk[:], kc[:, hl, bsl], i16[:])
                        kn = rpool.tile([128, 128], F16, tag="knat", bufs=4)
                        nc.scalar.mul(kn[:], tpk[:],
                                      vf[:, blk01, ch, hl:hl + 1])
                        knat.append(kn)
                at0ps = ps_small([128, 256])
                nc.tensor.matmul(at0ps[:], kc[:, hl, 0:128], qc[:, hl, :],
                                 start=True, stop=True)
                at0 = rpool.tile([128, CS], F16, tag="at0")
                nc.vector.scalar_tensor_tensor(
                    at0[:, 0:128], at0ps[:, 0:128],
                    vf[:, 0, ch, hl:hl + 1], Um32,
                    op0=ALU.mult, op1=ALU.mult)
                nc.scalar.mul(at0[:, 128:256], at0ps[:, 128:256],
                              vf[:, 0, ch, hl:hl + 1])
                at1ps = ps_small([128, 128])
                nc.tensor.matmul(at1ps[:], kc[:, hl, 128:256],
                                 qc[:, hl, 128:256], start=True, stop=True)
                at1 = rpool.tile([128, 128], F16, tag="at1s")
                nc.vector.scalar_tensor_tensor(
                    at1[:], at1ps[:], vf[:, 1, ch, hl:hl + 1], Um32,
                    op0=ALU.mult, op1=ALU.mult)
                return knat, at0, at1

            def emit_consume(ch, hl, pr, qc, vcn, o_st):
                knat, at0, at1 = pr
                # o = intra + inter (one PSUM group per ci half)
                for ci in range(2):
                    csl = slice(ci * 128, ci * 128 + 128)
                    mms = [(at0[:, csl], vcn[0][:, hl * 128:(hl + 1) * 128])]
                    if ci == 1:
                        mms.append((at1[:],
                                    vcn[1][:, hl * 128:(hl + 1) * 128]))
                    if ch > 0:
                        mms.append((qc[:, hl, csl], S_prev[hl][:]))
                    ops = ps_small([128, HD])
                    for i, (lh, rh) in enumerate(mms):
                        nc.tensor.matmul(ops[:], lh, rh, start=(i == 0),
                                         stop=(i == len(mms) - 1))
                    nc.scalar.mul(o_st[:, ci * HPC + hl, :], ops[:],
                                  rf[:, ci, ch, hl:hl + 1])
                # state update: S_cur = (S_prev + (vf*k)^T v) * eS
                if ch < NCH - 1:
                    sps = ps_small([128, HD])
                    nc.tensor.matmul(sps[:], knat[0][:],
                                     vcn[0][:, hl * 128:(hl + 1) * 128],
                                     start=True, stop=False)
                    nc.tensor.matmul(sps[:], knat[1][:],
                                     vcn[1][:, hl * 128:(hl + 1) * 128],
                                     start=False, stop=(ch == 0))
                    if ch > 0:
                        # += S_prev on the PE via identity stationary
                        nc.tensor.matmul(sps[:], i16, S_prev[hl][:],
                                         start=False, stop=True)
                    S_cur = rpool.tile([128, HD], F16, tag=f"S{hl}")
                    nc.vector.tensor_scalar(
                        S_cur[:], sps[:], eS[:, ch, hl:hl + 1], None,
                        ALU.mult)
                    S_prev[hl] = S_cur

            def emit_norm(ch, o_st):
                # o-norm over head dim (free axis)
                osq = rpool.tile([128, 2 * HPC, HD], F16, tag="osq", bufs=1)
                ssum = rpool.tile([128, 2 * HPC], F32, tag="ossum")
                nc.scalar.activation(osq[:], o_st[:], ACTF.Square)
                nc.vector.tensor_reduce(ssum[:], osq[:], AX.X, ALU.add)
                nc.vector.tensor_scalar(ssum[:], ssum[:], 1.0 / HD, EPS,
                                        ALU.mult, ALU.add)
                nc.vector.reciprocal(ssum[:], ssum[:])
                nc.scalar.activation(ssum[:], ssum[:], ACTF.Sqrt)
                o_n = rpool.tile([128, 2 * HPC, HD], F16, tag="o_n", bufs=2)
                nc.vector.tensor_tensor(
                    o_n[:], o_st[:],
                    ssum[:].unsqueeze(2).to_broadcast([128, 2 * HPC, HD]),
                    ALU.mult)
                return o_n

            def emit_outproj_half(ch, o_n, sg, go_st, m01):
                # gates for token-block m01, then its out-proj matmuls; the
                # other half's gate products land on vector while the PE runs
                # this half's matmuls, so the PE never waits on gating
                msl = slice(m01 * 128, m01 * 128 + 128)
                for hl in range(HPC):
                    trp = ps_small([128, 128], F16)
                    nc.tensor.transpose(
                        trp[:], o_n[:][:, m01 * HPC + hl, :], i16[:])
                    nc.vector.tensor_mul(
                        go_st[:, hl, msl], trp[:], sg[m01][:, hl, :])
                for n in range(DIM // 512):
                    ps = ps_big()
                    nsl = slice(n * 512, (n + 1) * 512)
                    for k in range(HPC):
                        nc.tensor.matmul(ps[:], go_st[:, k, msl],
                                         wo_sb[:, k, nsl],
                                         start=(k == 0), stop=(k == HPC - 1))
                    oo = epool.tile([128, 512], F16, tag="oo", bufs=4)
                    if n % 2 == 0:
                        nc.vector.tensor_copy(oo[:], ps[:])
                        nc.sync.dma_start(
                            out[ch * CS + m01 * 128:
                                ch * CS + m01 * 128 + 128, nsl], oo[:])
                    else:
                        nc.scalar.copy(oo[:], ps[:])
                        nc.scalar.dma_start(
                            out[ch * CS + m01 * 128:
                                ch * CS + m01 * 128 + 128, nsl], oo[:])

            def chunk_stream():
                """Retention pieces, one yield per piece; yields the chunk id
                of the NEXT piece so the driver can rate-limit. P2a/scales for
                the second half are injected at their emission-safe points."""
                prev = None
                for ch in range(NCH):
                    if ch == 7:
                        p2a_half(1)
                    if ch == 8:
                        # safe: caps keep chunks 6+ out of the tile loop, so
                        # this lands in the drain, after fire_allreduce(1)
                        scales_half(1)
                    yield ch
                    qc, kc, vcn, sg = emit_loads(ch)
                    o_st = rpool.tile([128, 2 * HPC, HD], F32, tag="o_st")
                    pr = None
                    for hl in range(HPC + 1):
                        yield ch
                        nxt_pr = (emit_produce(ch, hl, qc, kc)
                                  if hl < HPC else None)
                        if pr is not None:
                            emit_consume(ch, hl - 1, pr, qc, vcn, o_st)
                        pr = nxt_pr
                    if prev is not None:
                        go_st = rpool.tile([128, HPC, CS], F16, tag="go_st")
                        yield ch
                        emit_outproj_half(prev[0], prev[1], prev[2], go_st, 0)
                        yield ch
                        emit_outproj_half(prev[0], prev[1], prev[2], go_st, 1)
                    yield ch
                    o_n = emit_norm(ch, o_st)
                    prev = (ch, o_n, sg)
                yield NCH
                go_st = rpool.tile([128, HPC, CS], F16, tag="go_st")
                emit_outproj_half(prev[0], prev[1], prev[2], go_st, 0)
                emit_outproj_half(prev[0], prev[1], prev[2], go_st, 1)

            # ---- unified driver: tiles 0-7, retention interleaved 5-7 ----
            cs = chunk_stream()
            nxt = [next(cs)]

            def pump_one(cap):
                if nxt[0] is not None and nxt[0] <= cap:
                    nxt[0] = next(cs, None)

            for n in range(NT):
                xts = xt0 if n == 0 else load_x_tile(n)
                for _ in proj_tile(n, xts):
                    if n >= 5:
                        pump_one(2 * (n - 4) - 1)
                if n == 3:
                    fire_allreduce(0)
                if n == 4:
                    p2a_half(0)
                    scales_half(0)
            while nxt[0] is not None:
                nxt[0] = next(cs, None)

    nc.compile()
    return nc


def _prep_inputs(x, c, Wq, Wk, Wv, Wg, Wgt, Wo):
    """Build the 8 per-core input maps (host-side sharding / layout)."""
    consts = np.ascontiguousarray(_consts_np())
    c16 = np.concatenate([np.eye(128, dtype=np.float16),
                          np.ones((128, 8), np.float16)], axis=1)
    in_maps = []
    xTs = [np.ascontiguousarray(x[b].T).astype(np.float16) for b in range(B)]
    xcTs = [np.ascontiguousarray((x[b] + c[b]).T).astype(np.float16)
            for b in range(B)]
    for core in range(NCORE):
        b, g = core // 4, core % 4
        cols = slice(g * PCOLS, (g + 1) * PCOLS)
        heads = slice(g * HPC, (g + 1) * HPC)
        in_maps.append({
            "xT": xTs[b],
            "xcT": xcTs[b],
            "wq": np.ascontiguousarray(Wq[:, cols]).astype(np.float16),
            "wk": np.ascontiguousarray(Wk[:, cols]).astype(np.float16),
            "wv": np.ascontiguousarray(Wv[:, cols]).astype(np.float16),
            "wg": np.ascontiguousarray(Wg[:, cols]).astype(np.float16),
            "wgt": np.ascontiguousarray(Wgt[:, heads]).astype(np.float16),
            "wo": np.ascontiguousarray(Wo[cols, :]).astype(np.float16),
            "consts": consts,
            "c16": c16,
        })
    return in_maps


def kernel(x, c, Wq, Wk, Wv, Wg, Wgt, Wo, _want_results=False):
    key = "nc_dbg" if DEBUG else "nc"
    if key not in _cache:
        _cache[key] = build(debug=DEBUG)
    nc = _cache[key]
    in_maps = _prep_inputs(np.asarray(x, np.float32), np.asarray(c, np.float32),
                           np.asarray(Wq, np.float32), np.asarray(Wk, np.float32),
                           np.asarray(Wv, np.float32), np.asarray(Wg, np.float32),
                           np.asarray(Wgt, np.float32), np.asarray(Wo, np.float32))
    res = bass_utils.run_bass_kernel_spmd(
        nc, in_maps, core_ids=list(range(NCORE)), trace=TRACE)
    out = np.zeros((B, T, DIM), np.float32)
    for core in range(NCORE):
        out[core // 4] += res.results[core]["out"].astype(np.float32)
    if _want_results:
        return out, res
    return out


# revision 31
# speedup vs baseline: 1.4036x; 1.0100x over previous
"""GateRetention Trainium2 kernel (Bass/Tile), 8-core tensor-parallel.

Sharding: core grid (batch b = core//4, head-group g = core%4); each core owns
4 heads (512 cols of the q/k/v/g projections, 512 rows of Wo) of one batch.
RMS-norm statistics are AllReduced across each batch's 4 cores (two half-T
collectives so the latency hides under projection compute); out-proj partials
are summed on the host (row-parallel TP gather).

Pipeline: one merged projection pass over x (q,k,v,g,gt per 512-token tile,
fp16 operands, fp32 PSUM), fp16 staging via DRAM, then retention software-
pipelined by one chunk: retention(ch) overlaps the o-norm chain of ch-1 on
scalar/vector and the out-proj matmuls of ch-1 on PE. Elementwise work in
retention is spread over vector/scalar/gpsimd so no single engine stalls PE.

Precision: all matmuls fp16 with fp32 accumulation; a 2^-2 exponent shift on
vfac/rowfac keeps decayed v tiles inside fp16 range. x, x+c, and all weights
are cast to fp16 on the host.

kernel(**inputs) takes the FULL inputs from reference.setup_inputs() and
returns the FULL [B, T, DIM] fp32 output.
"""
import os
import sys

sys.path.insert(0, "/opt/trn_rl_repo")

import numpy as np

import concourse.bass as bass
import concourse.bacc as bacc
import concourse.tile as tile
import concourse.mybir as mybir
from concourse import bass_utils

F32 = mybir.dt.float32
F32R = mybir.dt.float32r
F16 = mybir.dt.float16
AX = mybir.AxisListType
ALU = mybir.AluOpType
ACTF = mybir.ActivationFunctionType

B, T, DIM = 2, 4096, 2048
H, HD = 16, 128
CS = 256
NCH = T // CS              # 16 chunks
EPS = 1e-5
GLN = 16.0
SCALE = HD ** -0.5
NCORE = 8
HPC = 4                    # heads per core
PCOLS = HPC * HD           # 512 cols per core
NBLK = T // 128            # 32 token blocks of 128
NT = T // 512              # 8 token n-tiles
VSH = 2.0 ** -2            # fp16 range shift on vv; inverse folded into rowfac

DEBUG_LVL = int(os.environ.get("GR_DEBUG", "0"))
DEBUG = bool(DEBUG_LVL)
TRACE = bool(int(os.environ.get("GR_TRACE", "0")))

_cache = {}


def _consts_np():
    """[128, 650] fp32: identity | Lm | Om | Um | ones | Lc | biases."""
    ident = np.eye(128, dtype=np.float32)
    jj, ii = np.meshgrid(np.arange(128), np.arange(128), indexing="ij")
    Lm = np.where(jj <= ii, -1.0 / GLN, 0.0).astype(np.float32)
    Om = np.full((128, 128), -1.0 / GLN, np.float32)
    Um = np.where(jj <= ii, 1.0, 0.0).astype(np.float32)
    ones = np.ones((128, 8), np.float32)
    # Lc: b_i - b_mid for block0 = +1/GLN * sum_{j>i} sp_j
    Lc = np.where(jj > ii, 1.0 / GLN, 0.0).astype(np.float32)
    bias = np.tile(np.array([[LN_RF, LN_VF]], np.float32), (128, 1))
    return np.concatenate([ident, Lm, Om, Um, ones, Lc, bias], axis=1)


def build(debug=False):
    nc = bacc.Bacc("TRN2", target_bir_lowering=False, debug=False,
                   enable_asserts=False, num_devices=NCORE)

    # ---------------- I/O ----------------
    xT = nc.dram_tensor("xT", [DIM, T], F16, kind="ExternalInput").ap()
    xcT = nc.dram_tensor("xcT", [DIM, T], F16, kind="ExternalInput").ap()
    wq = nc.dram_tensor("wq", [DIM, PCOLS], F16, kind="ExternalInput").ap()
    wk = nc.dram_tensor("wk", [DIM, PCOLS], F16, kind="ExternalInput").ap()
    wv = nc.dram_tensor("wv", [DIM, PCOLS], F16, kind="ExternalInput").ap()
    wg = nc.dram_tensor("wg", [DIM, PCOLS], F16, kind="ExternalInput").ap()
    wgt = nc.dram_tensor("wgt", [DIM, HPC], F16, kind="ExternalInput").ap()
    wo = nc.dram_tensor("wo", [PCOLS, DIM], F16, kind="ExternalInput").ap()
    consts = nc.dram_tensor("consts", [128, 650], F32R, kind="ExternalInput").ap()
    c16 = nc.dram_tensor("c16", [128, 136], F16, kind="ExternalInput").ap()
    out = nc.dram_tensor("out", [T, DIM], F16, kind="ExternalOutput").ap()

    def dbg(name, shape, dtype=F32):
        return nc.dram_tensor(name, shape, dtype, kind="ExternalOutput").ap()

    with tile.TileContext(nc) as tc:
        with (
            tc.tile_pool(name="const", bufs=1) as cpool,
            tc.tile_pool(name="wts", bufs=1) as wpool,
            tc.tile_pool(name="xstream", bufs=2) as xpool,
            tc.tile_pool(name="evac", bufs=2) as epool,
            tc.tile_pool(name="persist", bufs=1) as ppool,
            tc.tile_pool(name="small", bufs=2) as spool,
            tc.tile_pool(name="ret", bufs=2) as rpool,
            tc.tile_pool(name="ps", bufs=1, space="PSUM") as psp,
        ):
            def ps_big():
                return psp.tile([128, 512], F32, tag="big", bufs=4, name="psbig")

            def ps_small(shape=None, dtype=F32):
                return psp.tile(shape or [128, 256], dtype, tag="small", bufs=4,
                                name="pssmall")

            # ---------------- constants ----------------
            cst = cpool.tile([128, 650], F32R, tag="consts")
            nc.sync.dma_start(cst[:], consts)
            ident = cst[:, 0:128]
            ident32 = ident.bitcast(F32)
            Lm = cst[:, 128:256]
            Om = cst[:, 256:384]
            Um = cst[:, 384:512]
            Um32 = Um.bitcast(F32)
            ones1 = cst[:, 512:513]
            Lc = cst[:, 520:648]
            ln_rf = cst[:, 648:649].bitcast(F32)
            ln_vf = cst[:, 649:650].bitcast(F32)
            i16f = cpool.tile([128, 136], F16, tag="i16")
            nc.sync.dma_start(i16f[:], c16)
            i16 = i16f[:, 0:128]
            ones16 = i16f[:, 128:129]

            # ---------------- DRAM scratch (fp16) ----------------
            if debug:
                qT_s = dbg("dbg_qT", [PCOLS, T], F16)
                kT_s = dbg("dbg_kT", [PCOLS, T], F16)
                vN_s = dbg("dbg_vN", [T, PCOLS], F16)
                gT_s = dbg("dbg_gT", [PCOLS, T], F16)
            else:
                qT_s = nc.dram_tensor("qT_s", [PCOLS, T], F16,
                                      kind="Internal").ap()
                kT_s = nc.dram_tensor("kT_s", [PCOLS, T], F16,
                                      kind="Internal").ap()
                vN_s = nc.dram_tensor("vN_s", [T, PCOLS], F16,
                                      kind="Internal").ap()
                gT_s = nc.dram_tensor("gT_s", [PCOLS, T], F16,
                                      kind="Internal").ap()
            ss_in = [nc.dram_tensor(f"ss_in{h}", [3, T // 2], F32,
                                    kind="Internal").ap() for h in range(2)]
            ss_out = [nc.dram_tensor(f"ss_out{h}", [3, T // 2], F32,
                                     kind="Internal").ap() for h in range(2)]

            # =========================================================
            # P1: merged projection pass, fp16 math, fp16 staging
            # =========================================================
            gtn = ppool.tile([128, 2, NBLK // 2, HPC], F32, tag="gtn")
            vss = ppool.tile([128, NBLK], F32, tag="vss")

            # weights split per k-tile so the first matmuls start after
            # a single 128-row slice lands (kills the startup DMA stall)
            def load_w_split(wdram, tag):
                tiles = []
                for k in range(16):
                    wt = wpool.tile([128, PCOLS], F16, tag=f"{tag}{k}")
                    nc.sync.dma_start(wt[:], wdram[k * 128:(k + 1) * 128, :])
                    tiles.append(wt)
                return tiles

            def load_x_tile(n):
                tok = slice(n * 512, (n + 1) * 512)
                xts = []
                for k in range(16):
                    xt = xpool.tile([128, 512], F16, tag=f"x{k}")
                    nc.gpsimd.dma_start(xt[:], xT[k * 128:(k + 1) * 128, tok])
                    xts.append(xt)
                return xts

            wq_t = load_w_split(wq, "wq")
            xt0 = load_x_tile(0)
            wk_t = load_w_split(wk, "wk")
            wv_t = load_w_split(wv, "wv")
            wg_t = load_w_split(wg, "wg")
            wgt_sb = wpool.tile([128, 16, HPC], F16, tag="wgt")
            nc.sync.dma_start(wgt_sb[:],
                              wgt.rearrange("(kt p) m -> p kt m", p=128))
            wo_sb = wpool.tile([128, HPC, DIM], F16, tag="wo")
            nc.sync.dma_start(wo_sb[:], wo.rearrange("(h p) m -> p h m", p=128))

            def stat_flush(pend):
                """Deferred sumsq: 4 ones-matmuls emitted one section later so
                the PE never waits on the scalar Square of a fresh PSUM."""
                if pend is None:
                    return
                sqts, pi, half, tl = pend
                ssps = ps_small([1, 512])
                for m in range(4):
                    nc.tensor.matmul(ssps[:1, :], ones16, sqts[m][:],
                                     start=(m == 0), stop=(m == 3))
                ssev = spool.tile([1, 512], F32, tag="ssev", bufs=2)
                nc.vector.tensor_copy(ssev[:], ssps[:1, :])
                nc.scalar.dma_start(ss_in[half][pi:pi + 1, tl], ssev[:])

            def gt_matmuls(n):
                tok = slice(n * 512, (n + 1) * 512)
                xc = []
                for h4 in range(4):
                    xct = xpool.tile([128, 4, 512], F16, tag="xc", bufs=4)
                    nc.gpsimd.dma_start(
                        xct[:], xcT[h4 * 512:(h4 + 1) * 512, tok].rearrange(
                            "(kt p) m -> p kt m", p=128))
                    xc.append(xct)
                gtps = ps_small([128, 512])
                for k in range(16):
                    nc.tensor.matmul(gtps[:HPC, :], wgt_sb[:, k, :],
                                     xc[k // 4][:, k % 4, :],
                                     start=(k == 0), stop=(k == 15))
                gstg = spool.tile([HPC, 512], F32, tag="gstg", bufs=2)
                nc.vector.tensor_copy(gstg[:], gtps[:HPC, :])
                return gstg

            def gt_transposes(n, gstg):
                for j in range(4):
                    b = n * 4 + j
                    tp = ps_small([128, HPC])
                    nc.tensor.matmul(tp[:], gstg[:, j * 128:(j + 1) * 128],
                                     ident32[:HPC, :HPC], is_transpose=True)
                    nc.vector.tensor_copy(gtn[:, b % 2, b // 2, :], tp[:])

            def proj_tile(n, xts, with_gt=True):
                tok = slice(n * 512, (n + 1) * 512)
                half, tl = n // 4, slice((n % 4) * 512, (n % 4) * 512 + 512)
                pend = None

                def qk_evac(ps, m, sdram, sqts):
                    msl = slice(m * 128, (m + 1) * 128)
                    ev = epool.tile([128, 512], F16, tag="ev", bufs=4)
                    sqt = epool.tile([128, 512], F16, tag="sq", bufs=8)
                    if m % 2 == 0:
                        nc.vector.tensor_copy(ev[:], ps[:])
                        nc.sync.dma_start(sdram[msl, tok], ev[:])
                    else:
                        nc.scalar.copy(ev[:], ps[:])
                        nc.gpsimd.dma_start(sdram[msl, tok], ev[:])
                    nc.scalar.activation(sqt[:], ps[:], ACTF.Square)
                    sqts.append(sqt)

                # q, k: T-layout staging + deferred sumsq rows
                for pi, (wt, sdram) in enumerate(((wq_t, qT_s), (wk_t, kT_s))):
                    sqts = []
                    if n == 0 and pi == 0:
                        # cold start: k-outer so the first matmul begins after
                        # one 128-row weight/x slice lands, not the full tile
                        pss = [ps_big() for _ in range(4)]
                        for k in range(16):
                            for m in range(4):
                                nc.tensor.matmul(
                                    pss[m][:], wt[k][:, m * 128:(m + 1) * 128],
                                    xts[k][:], start=(k == 0), stop=(k == 15),
                                    skip_group_check=True)
                            if k % 4 == 3:
                                yield
                        for m in range(4):
                            qk_evac(pss[m], m, sdram, sqts)
                    else:
                        for m in range(4):
                            ps = ps_big()
                            for k in range(16):
                                nc.tensor.matmul(ps[:], wt[k][:, m * 128:
                                                               (m + 1) * 128],
                                                 xts[k][:],
                                                 start=(k == 0), stop=(k == 15))
                            qk_evac(ps, m, sdram, sqts)
                            yield
                    stat_flush(pend)
                    pend = (sqts, pi, half, tl)
                # v natural + sumsq accum
                for mt in range(4):
                    msl = slice(mt * 128, (mt + 1) * 128)
                    ps = ps_big()
                    for k in range(16):
                        nc.tensor.matmul(ps[:], xts[k][:, msl], wv_t[k][:],
                                         start=(k == 0), stop=(k == 15))
                    if mt == 0:
                        stat_flush(pend)
                        pend = None
                    ev = epool.tile([128, 512], F16, tag="ev", bufs=4)
                    sqt = epool.tile([128, 512], F16, tag="sq", bufs=8)
                    nc.scalar.activation(
                        sqt[:], ps[:], ACTF.Square,
                        accum_out=vss[:, n * 4 + mt:n * 4 + mt + 1])
                    if mt % 2 == 0:
                        nc.vector.tensor_copy(ev[:], ps[:])
                        nc.sync.dma_start(
                            vN_s[n * 512 + mt * 128:n * 512 + (mt + 1) * 128, :],
                            ev[:])
                    else:
                        nc.scalar.copy(ev[:], ps[:])
                        nc.gpsimd.dma_start(
                            vN_s[n * 512 + mt * 128:n * 512 + (mt + 1) * 128, :],
                            ev[:])
                    yield
                if n == 7:
                    fire_allreduce(1)
                if with_gt:
                    gstg = gt_matmuls(n)
                    yield
                # silu(g), T-layout (gt transposes deferred past it)
                for m in range(4):
                    msl = slice(m * 128, (m + 1) * 128)
                    ps = ps_big()
                    for k in range(16):
                        nc.tensor.matmul(ps[:], wg_t[k][:, msl], xts[k][:],
                                         start=(k == 0), stop=(k == 15))
                    ev = epool.tile([128, 512], F16, tag="ev", bufs=4)
                    nc.scalar.activation(ev[:], ps[:], ACTF.Silu)
                    nc.scalar.dma_start(gT_s[msl, tok], ev[:])
                    yield
                if with_gt:
                    gt_transposes(n, gstg)
                yield

            def fire_allreduce(half):
                # v sumsq for this half: [128, 16] -> [16, 128] -> row
                vssT = ps_small([128, 128])
                nc.tensor.matmul(vssT[:16, :],
                                 vss[:, half * 16:(half + 1) * 16],
                                 ident32, is_transpose=True)
                vssev = spool.tile([16, 128], F32, tag="vssev", bufs=2)
                nc.vector.tensor_copy(vssev[:], vssT[:16, :])
                nc.scalar.dma_start(
                    ss_in[half][2:3, :].rearrange("a (b c) -> (a b) c", c=128),
                    vssev[:])
                nc.gpsimd.collective_compute(
                    "AllReduce", ALU.add,
                    replica_groups=[[0, 1, 2, 3], [4, 5, 6, 7]],
                    ins=[ss_in[half].opt()], outs=[ss_out[half].opt()],
                )

            for n in range(NT):
                xts = xt0 if n == 0 else load_x_tile(n)
                proj_tile(n, xts)
                if n == 3:
                    fire_allreduce(0)
            fire_allreduce(1)

            # =========================================================
            # P2a: gate decays (AllReduce-independent, PE + scalar)
            # =========================================================
            ssn = ppool.tile([128, NBLK, 3], F32, tag="ssn")
            rsn = ppool.tile([128, NBLK, 3], F32, tag="rsn")
            skv = ppool.tile([128, NBLK], F32, tag="skv")

            # gate decays: sp = softplus(-z) = ln(1 + exp(-z)); -1/GLN in Lm/Om
            gtd = ppool.tile([128, 2, NBLK // 2, HPC], F32R, tag="gtd")
            nc.scalar.activation(gtn[:], gtn[:], ACTF.Exp, scale=-1.0)
            nc.scalar.activation(gtd[:], gtn[:], ACTF.Ln, bias=1.0)

            # batched recentering: one matmul per triangular mask over all 16
            # chunks (even blocks with Lc, odd with Lm); eS accumulates the two
            # full-block sums (blocks 2ch+1, 2ch+2) in one pair of matmuls
            e_rf = ppool.tile([128, 2, NCH, HPC], F32, tag="erf")
            e_vf = ppool.tile([128, 2, NCH, HPC], F32, tag="evf")
            rf = ppool.tile([128, NCH, 2, HPC], F32, tag="rf")      # rowfac
            vf = ppool.tile([128, NCH, 2, HPC], F32, tag="vf")      # vfac
            eS = ppool.tile([128, NCH, HPC], F32, tag="eS")
            p0a = ps_small([128, NCH * HPC])
            nc.tensor.matmul(p0a[:], Lc, gtd[:, 0].rearrange("p a b -> p (a b)"),
                             start=True, stop=True)
            p1a = ps_small([128, NCH * HPC])
            nc.tensor.matmul(p1a[:], Lm, gtd[:, 1].rearrange("p a b -> p (a b)"),
                             start=True, stop=True)
            pta = ps_small([128, (NCH - 1) * HPC])
            nc.tensor.matmul(pta[:],
                             Om, gtd[:, 1, 0:NCH - 1].rearrange("p a b -> p (a b)"),
                             start=True, stop=False)
            nc.tensor.matmul(pta[:],
                             Om, gtd[:, 0, 1:NCH].rearrange("p a b -> p (a b)"),
                             start=False, stop=True)
            nc.scalar.activation(
                eS[:, 0:NCH - 1, :].rearrange("p a b -> p (a b)"), pta[:],
                ACTF.Exp)
            for blk01, bps in ((0, p0a), (1, p1a)):
                nc.scalar.activation(
                    e_rf[:, blk01].rearrange("p a b -> p (a b)"), bps[:],
                    ACTF.Exp)
                nc.scalar.activation(
                    e_vf[:, blk01].rearrange("p a b -> p (a b)"), bps[:],
                    ACTF.Exp, scale=-1.0)

            def scales_half(half):
                """AR-dependent: rsqrt of mean sumsq, then rf/vf for 8 chunks."""
                bsl = slice(half * 16, (half + 1) * 16)
                for nn_ in range(4):
                    tl = slice(nn_ * 512, (nn_ + 1) * 512)
                    srt = spool.tile([3, 512], F32, tag="srt", bufs=2)
                    nc.sync.dma_start(srt[:], ss_out[half][:, tl])
                    for j in range(4):
                        tp = ps_small([128, 4])
                        nc.tensor.matmul(tp[:, :3],
                                         srt[:, j * 128:(j + 1) * 128],
                                         ident32[:3, :3], is_transpose=True)
                        nc.vector.tensor_copy(
                            ssn[:, half * 16 + nn_ * 4 + j, :], tp[:, :3])
                nc.vector.tensor_scalar(rsn[:, bsl], ssn[:, bsl], 1.0 / DIM,
                                        EPS, ALU.mult, ALU.add)
                nc.scalar.activation(rsn[:, bsl], rsn[:, bsl], ACTF.Ln)
                nc.scalar.activation(rsn[:, bsl], rsn[:, bsl], ACTF.Exp,
                                     scale=-0.5)
                nc.vector.tensor_mul(skv[:, bsl], rsn[:, bsl, 1],
                                     rsn[:, bsl, 2])
                for ch in range(half * 8, half * 8 + 8):
                    for blk01 in range(2):
                        blk = 2 * ch + blk01
                        nc.vector.tensor_scalar(
                            rf[:, ch, blk01, :], e_rf[:, blk01, ch, :],
                            rsn[:, blk, 0:1], SCALE / VSH, ALU.mult, ALU.mult)
                        nc.vector.tensor_scalar(
                            vf[:, ch, blk01, :], e_vf[:, blk01, ch, :],
                            skv[:, blk:blk + 1], VSH, ALU.mult, ALU.mult)

            scales_half(0)

            if debug and DEBUG_LVL >= 2:
                nc.sync.dma_start(dbg("dbg_rsn", [128, NBLK * 3]),
                                  rsn[:].rearrange("p a b -> p (a b)"))
                nc.sync.dma_start(
                    dbg("dbg_gtd", [128, NBLK * HPC]),
                    gtd[:].bitcast(F32).rearrange("p a b c -> p (a b c)"))

            # =========================================================
            # P3: retention, software-pipelined by one chunk
            # =========================================================
            S_prev = [None] * HPC

            def emit_loads(ch):
                tok = slice(ch * CS, (ch + 1) * CS)
                qc = rpool.tile([128, HPC, CS], F16, tag="qc", bufs=2)
                kc = rpool.tile([128, HPC, CS], F16, tag="kc", bufs=2)
                for t_, s_ in ((qc, qT_s), (kc, kT_s)):
                    nc.sync.dma_start(
                        t_[:], s_[:, tok].rearrange("(h p) m -> p h m", p=128))
                vcn, sg = [], []
                for blk01 in range(2):
                    bt = slice(ch * CS + blk01 * 128, ch * CS + blk01 * 128 + 128)
                    vt = rpool.tile([128, PCOLS], F16, tag="vcn", bufs=4)
                    nc.sync.dma_start(vt[:], vN_s[bt, :])
                    vcn.append(vt)
                    gt_ = rpool.tile([128, HPC, 128], F16, tag="gch", bufs=4)
                    nc.sync.dma_start(
                        gt_[:], gT_s[:, bt].rearrange("(h p) m -> p h m", p=128))
                    sg.append(gt_)
                return qc, kc, vcn, sg

            def emit_produce(ch, hl, qc, kc):
                """Per-head producers: k transposes + AT matmuls (PE) and
                their fp16 evacs with vfac folded in (scalar/vector). Runs one
                head ahead of emit_consume so PE never waits on the evacs."""
                knat = []
                for blk01 in range(2):
                    bsl = slice(blk01 * 128, blk01 * 128 + 128)
                    if ch < NCH - 1:
                        tpk = ps_small([128, 128], F16)
                        nc.tensor.transpose(tpk[:], kc[:, hl, bsl], i16[:])
                        kn = rpool.tile([128, 128], F16, tag="knat", bufs=6)
                        nc.scalar.mul(kn[:], tpk[:],
                                      vf[:, blk01, ch, hl:hl + 1])
                        knat.append(kn)
                at0ps = ps_small([128, 256])
                nc.tensor.matmul(at0ps[:], kc[:, hl, 0:128], qc[:, hl, :],
                                 start=True, stop=True)
                at0 = rpool.tile([128, CS], F16, tag="at0", bufs=3)
                nc.vector.scalar_tensor_tensor(
                    at0[:, 0:128], at0ps[:, 0:128],
                    vf[:, 0, ch, hl:hl + 1], Um32,
                    op0=ALU.mult, op1=ALU.mult)
                nc.scalar.mul(at0[:, 128:256], at0ps[:, 128:256],
                              vf[:, 0, ch, hl:hl + 1])
                at1ps = ps_small([128, 128])
                nc.tensor.matmul(at1ps[:], kc[:, hl, 128:256],
                                 qc[:, hl, 128:256], start=True, stop=True)
                at1 = rpool.tile([128, 128], F16, tag="at1s", bufs=3)
                nc.vector.scalar_tensor_tensor(
                    at1[:], at1ps[:], vf[:, 1, ch, hl:hl + 1], Um32,
                    op0=ALU.mult, op1=ALU.mult)
                return knat, at0, at1

            def emit_consume(ch, hl, pr, qc, vcn, o_st):
                knat, at0, at1 = pr
                # o = intra + inter (one PSUM group per ci half)
                for ci in range(2):
                    csl = slice(ci * 128, ci * 128 + 128)
                    mms = [(at0[:, csl], vcn[0][:, hl * 128:(hl + 1) * 128])]
                    if ci == 1:
                        mms.append((at1[:],
                                    vcn[1][:, hl * 128:(hl + 1) * 128]))
                    if ch > 0:
                        mms.append((qc[:, hl, csl], S_prev[hl][:]))
                    ops = ps_small([128, HD])
                    for i, (lh, rh) in enumerate(mms):
                        nc.tensor.matmul(ops[:], lh, rh, start=(i == 0),
                                         stop=(i == len(mms) - 1))
                    nc.scalar.mul(o_st[:, ci * HPC + hl, :], ops[:],
                                  rf[:, ci, ch, hl:hl + 1])
                # state update: S_cur = (S_prev + (vf*k)^T v) * eS
                if ch < NCH - 1:
                    sps = ps_small([128, HD])
                    nc.tensor.matmul(sps[:], knat[0][:],
                                     vcn[0][:, hl * 128:(hl + 1) * 128],
                                     start=True, stop=False)
                    nc.tensor.matmul(sps[:], knat[1][:],
                                     vcn[1][:, hl * 128:(hl + 1) * 128],
                                     start=False, stop=(ch == 0))
                    if ch > 0:
                        # += S_prev on the PE via identity stationary
                        nc.tensor.matmul(sps[:], i16, S_prev[hl][:],
                                         start=False, stop=True)
                    S_cur = rpool.tile([128, HD], F16, tag=f"S{hl}")
                    nc.vector.tensor_scalar(
                        S_cur[:], sps[:], eS[:, ch, hl:hl + 1], None,
                        ALU.mult)
                    S_prev[hl] = S_cur

            def emit_norm(ch, o_st):
                # o-norm over head dim (free axis)
                osq = rpool.tile([128, 2 * HPC, HD], F16, tag="osq", bufs=1)
                ssum = rpool.tile([128, 2 * HPC], F32, tag="ossum")
                nc.scalar.activation(osq[:], o_st[:], ACTF.Square)
                nc.vector.tensor_reduce(ssum[:], osq[:], AX.X, ALU.add)
                nc.vector.tensor_scalar(ssum[:], ssum[:], 1.0 / HD, EPS,
                                        ALU.mult, ALU.add)
                nc.vector.reciprocal(ssum[:], ssum[:])
                nc.scalar.activation(ssum[:], ssum[:], ACTF.Sqrt)
                o_n = rpool.tile([128, 2 * HPC, HD], F16, tag="o_n", bufs=2)
                nc.vector.tensor_tensor(
                    o_n[:], o_st[:],
                    ssum[:].unsqueeze(2).to_broadcast([128, 2 * HPC, HD]),
                    ALU.mult)
                return o_n

            def emit_outproj_gates(ch, o_n, sg):
                # transpose + gate into go_st
                go_st = rpool.tile([128, HPC, CS], F16, tag="go_st")
                for hl in range(HPC):
                    for blk01 in range(2):
                        trp = ps_small([128, 128], F16)
                        nc.tensor.transpose(
                            trp[:], o_n[:][:, blk01 * HPC + hl, :], i16[:])
                        bsl = slice(blk01 * 128, blk01 * 128 + 128)
                        nc.vector.tensor_mul(
                            go_st[:, hl, bsl], trp[:], sg[blk01][:, hl, :])
                return go_st

            def emit_outproj_mm(ch, go_st):
                for m01 in range(2):
                    msl = slice(m01 * 128, m01 * 128 + 128)
                    for n in range(DIM // 512):
                        ps = ps_big()
                        nsl = slice(n * 512, (n + 1) * 512)
                        for k in range(HPC):
                            nc.tensor.matmul(ps[:], go_st[:, k, msl],
                                             wo_sb[:, k, nsl],
                                             start=(k == 0), stop=(k == HPC - 1))
                        oo = epool.tile([128, 512], F16, tag="oo", bufs=4)
                        if n % 2 == 0:
                            nc.vector.tensor_copy(oo[:], ps[:])
                            nc.sync.dma_start(
                                out[ch * CS + m01 * 128:
                                    ch * CS + m01 * 128 + 128, nsl], oo[:])
                        else:
                            nc.scalar.copy(oo[:], ps[:])
                            nc.scalar.dma_start(
                                out[ch * CS + m01 * 128:
                                    ch * CS + m01 * 128 + 128, nsl], oo[:])

            def chunk_stream():
                """Retention pieces, one yield per piece; yields the chunk id
                of the NEXT piece so the driver can rate-limit. P2a/scales for
                the second half are injected at their emission-safe points."""
                prev = None
                for ch in range(NCH):
                    if ch == 7:
                        p2a_half(1)
                    if ch == 8:
                        # safe: caps keep chunks 6+ out of the tile loop, so
                        # this lands in the drain, after fire_allreduce(1)
                        scales_half(1)
                    yield ch
                    qc, kc, vcn, sg = emit_loads(ch)
                    o_st = rpool.tile([128, 2 * HPC, HD], F32, tag="o_st")
                    prs = {}
                    for i in range(HPC + 2):
                        yield ch
                        if i < HPC:
                            prs[i] = emit_produce(ch, i, qc, kc)
                        if i >= 2:
                            emit_consume(ch, i - 2, prs.pop(i - 2), qc, vcn,
                                         o_st)
                    if prev is not None:
                        yield ch
                        go_st = emit_outproj_gates(prev[0], prev[1], prev[2])
                        yield ch
                        emit_outproj_mm(prev[0], go_st)
                    yield ch
                    o_n = emit_norm(ch, o_st)
                    prev = (ch, o_n, sg)
                yield NCH
                go_st = emit_outproj_gates(prev[0], prev[1], prev[2])
                emit_outproj_mm(prev[0], go_st)

            # ---- unified driver: tiles 0-7, retention interleaved 5-7 ----
            cs = chunk_stream()
            nxt = [next(cs)]

            def pump_one(cap):
                if nxt[0] is not None and nxt[0] <= cap:
                    nxt[0] = next(cs, None)

            for n in range(NT):
                xts = xt0 if n == 0 else load_x_tile(n)
                for _ in proj_tile(n, xts):
                    if n >= 5:
                        pump_one(2 * (n - 4) - 1)
                if n == 3:
                    fire_allreduce(0)
                if n == 4:
                    p2a_half(0)
                    scales_half(0)
            while nxt[0] is not None:
                nxt[0] = next(cs, None)

    nc.compile()
    return nc


def _prep_inputs(x, c, Wq, Wk, Wv, Wg, Wgt, Wo):
    """Build the 8 per-core input maps (host-side sharding / layout)."""
    consts = np.ascontiguousarray(_consts_np())
    c16 = np.concatenate([np.eye(128, dtype=np.float16),
                          np.ones((128, 8), np.float16)], axis=1)
    in_maps = []
    xTs = [np.ascontiguousarray(x[b].T).astype(np.float16) for b in range(B)]
    xcTs = [np.ascontiguousarray((x[b] + c[b]).T).astype(np.float16)
            for b in range(B)]
    for core in range(NCORE):
        b, g = core // 4, core % 4
        cols = slice(g * PCOLS, (g + 1) * PCOLS)
        heads = slice(g * HPC, (g + 1) * HPC)
        in_maps.append({
            "xT": xTs[b],
            "xcT": xcTs[b],
            "wq": np.ascontiguousarray(Wq[:, cols]).astype(np.float16),
            "wk": np.ascontiguousarray(Wk[:, cols]).astype(np.float16),
            "wv": np.ascontiguousarray(Wv[:, cols]).astype(np.float16),
            "wg": np.ascontiguousarray(Wg[:, cols]).astype(np.float16),
            "wgt": np.ascontiguousarray(Wgt[:, heads]).astype(np.float16),
            "wo": np.ascontiguousarray(Wo[cols, :]).astype(np.float16),
            "consts": consts,
            "c16": c16,
        })
    return in_maps


def kernel(x, c, Wq, Wk, Wv, Wg, Wgt, Wo, _want_results=False):
    key = "nc_dbg" if DEBUG else "nc"
    if key not in _cache:
        _cache[key] = build(debug=DEBUG)
    nc = _cache[key]
    in_maps = _prep_inputs(np.asarray(x, np.float32), np.asarray(c, np.float32),
                           np.asarray(Wq, np.float32), np.asarray(Wk, np.float32),
                           np.asarray(Wv, np.float32), np.asarray(Wg, np.float32),
                           np.asarray(Wgt, np.float32), np.asarray(Wo, np.float32))
    res = bass_utils.run_bass_kernel_spmd(
        nc, in_maps, core_ids=list(range(NCORE)), trace=TRACE)
    out = np.zeros((B, T, DIM), np.float32)
    for core in range(NCORE):
        out[core // 4] += res.results[core]["out"].astype(np.float32)
    if _want_results:
        return out, res
    return out
